# revision 30
# baseline (speedup 1.0000x reference)
"""Multi-layer bidirectional Tree-LSTM on 8 TRN2 NeuronCores.

Strategy: the input is a complete binary tree of 1024 nodes. Below level 3
there are 8 independent subtrees (rooted at nodes 7..14) -> one subtree per
core (data parallel). The top 7 nodes (0..6) are computed replicated on all
cores; one small AllGather per layer exchanges the 8 subtree-root (h, c)
pairs for the leaves->root direction.

On-device layout is feature-major (hidden dim on partitions, nodes on the
free axis); weights are stationary and node columns stream, so no
transposes are needed anywhere in the recurrence.

Per-core column layout (143 columns):
  0..126   : BFS slots of subtree(7+c)  (slot s, level k=floor(log2(s+1)))
  127      : node 1023 (replicated on every core; only core 0's is used)
  128..134 : top nodes 0..6 (replicated)
  135..142 : subtree roots 7..14 (fwd: from AllGather; bwd: replicated)

Precision: pre-projection weights bf16; recurrence weights fp8 e4m3
scaled by 64 (the recurrence is LDWEIGHTS-bandwidth-bound and fp8 FWL
loads 2x faster than bf16; the 1/64 un-scale is folded into the bf16
child/parent-h copies, which is a lossless exponent shift). Measured
end-to-end rel err ~6e-3 vs the 2e-2 gate.

Gate row order is permuted at pack time so all sigmoid gates are
contiguous: fwd [ig og fl fr r u] (sigmoid 0:20, tanh 20:24), bwd
[ig og f r u] (sigmoid 0:16, tanh 16:20).

All weight DRAM tensors are host-packed to the exact SBUF layout so every
weight DMA is a plain contiguous [128, X] copy (large descriptors, full
HBM bandwidth). The layer pipeline enqueues layer l+1's full weight
stream on the SP DMA ring BEFORE layer l's recurrence emits its
collective-dependent DMAs, so the next layer's weights prefetch during
the recurrence instead of queueing behind the AllGather.
"""

import os
import sys

for _p in ("/opt/trn_rl_repo",):
    if _p not in sys.path and os.path.isdir(_p):
        sys.path.insert(0, _p)

import numpy as np
import ml_dtypes

try:
    import jax
    jax.config.update("jax_compilation_cache_dir", os.environ.get("KERNEL_JAX_CACHE", "/tmp/jax_neff_cache"))
    jax.config.update("jax_persistent_cache_min_compile_time_secs", 5.0)
    jax.config.update("jax_persistent_cache_min_entry_size_bytes", 0)
except Exception:
    pass

import concourse.bass as bass
import concourse.mybir as mybir
from concourse import bacc
from concourse.tile import TileContext
from concourse.bass_utils import run_bass_kernel_spmd

BF16 = ml_dtypes.bfloat16
FP8 = ml_dtypes.float8_e4m3fn
F32 = mybir.dt.float32
B16 = mybir.dt.bfloat16
F8 = mybir.dt.float8e4
AF = mybir.ActivationFunctionType

N, D, H, L = 1024, 1024, 512, 2
NCOL = 143  # 127 subtree + node1023 + 7 top + 8 roots
NCORES = 8
# fp8 recurrence scales: weights x64, h operand x4. The product (x256) is
# folded into the pre-projection gate rows/biases on the host, and undone
# by the activation `scale` parameter (powers of two -> exact).
WSCALE = 64.0
HSCALE = 4.0
GSCALE = WSCALE * HSCALE
GINV = 1.0 / GSCALE
# pre-projection group streaming order: PRE_B groups first
GORDER = [7, 8, 9, 10, 11, 12, 0, 1, 2, 3, 4, 5, 6]

_last_results = None  # stashed BassKernelResults for test.py


def _node_ids(c):
    ids = []
    for k in range(7):
        base = (8 + c) * (1 << k) - 1
        ids.extend(range(base, base + (1 << k)))
    ids.append(1023)
    ids.extend(range(0, 7))
    ids.extend(range(7, 15))
    return np.asarray(ids, dtype=np.int64)


def _pack_lhsT(w, kchunks, mchunks, dty):
    # w: [M, K] fp32 -> lhsT tiles [kchunks, mchunks, 128, 128] where
    # tile[k, m, kp, mc] = w[m*128+mc, k*128+kp]
    Mdim, Kdim = w.shape
    assert Mdim == mchunks * 128 and Kdim == kchunks * 128
    t = w.reshape(mchunks, 128, kchunks, 128).transpose(2, 0, 3, 1)
    return np.ascontiguousarray(t.astype(dty))


def _perm_f(w):
    # fwd gate rows [ig og fl fr u r] -> [ig og fl fr r u]
    return np.concatenate([w[0:2048], w[2560:3072], w[2048:2560]], axis=0)


def _perm_b(w):
    # bwd gate rows [ig og f u r] -> [ig og f r u]
    return np.concatenate([w[0:1536], w[2048:2560], w[1536:2048]], axis=0)


def _build_program():
    nc = bacc.Bacc("TRN2", target_bir_lowering=False, debug=False,
                   num_devices=NCORES)

    featsT_d = nc.dram_tensor("featsT", [128, 8 * NCOL], B16, kind="ExternalInput")
    wpre_d, wrecf_d, wrecb_d, biasf_d, biasb_d = [], [], [], [], []
    for l in range(L):
        wpre_d.append(nc.dram_tensor(f"wpre{l}", [13, 128, 4096], B16,
                                     kind="ExternalInput"))
        wrecf_d.append(nc.dram_tensor(f"wrecf{l}", [128, 8 * 24 * 128], F8,
                                      kind="ExternalInput"))
        wrecb_d.append(nc.dram_tensor(f"wrecb{l}", [128, 4 * 20 * 128], F8,
                                      kind="ExternalInput"))
        biasf_d.append(nc.dram_tensor(f"biasf{l}", [128, 28], F32,
                                      kind="ExternalInput"))
        biasb_d.append(nc.dram_tensor(f"biasb{l}", [128, 24], F32,
                                      kind="ExternalInput"))
    mask_d = nc.dram_tensor("mask", [128, 1], F32, kind="ExternalInput")
    psel_d = nc.dram_tensor("psel", [128, 8], F32, kind="ExternalInput")
    out_loc_d = nc.dram_tensor("out_loc", [1024, 128], F32, kind="ExternalOutput")
    out_top_d = nc.dram_tensor("out_top", [1024, 7], F32, kind="ExternalOutput")

    with TileContext(nc) as tc:
        with (
            tc.tile_pool(name="state", bufs=1) as state_p,
            tc.tile_pool(name="weights", bufs=2) as w_p,
            tc.tile_pool(name="pre", bufs=1) as pre_p,
            tc.tile_pool(name="wstream", bufs=9) as ws_p,
            tc.tile_pool(name="scratch", bufs=2) as sc_p,
            tc.tile_pool(name="psum_pre", bufs=2, space="PSUM") as pp_p,
            tc.tile_pool(name="psum_rec", bufs=2, space="PSUM") as pr_p,
            tc.tile_pool(name="dram", bufs=1, space="DRAM") as dram_p,
        ):
            HF = state_p.tile([128, 4, NCOL], F32, name="HF")
            CF = state_p.tile([128, 4, NCOL], F32, name="CF")
            HB = state_p.tile([128, 4, NCOL], F32, name="HB")
            CB = state_p.tile([128, 4, NCOL], F32, name="CB")
            mask_sb = state_p.tile([128, 1], F32, name="mask_sb")
            psel_sb = state_p.tile([128, 8], F32, name="psel_sb")
            nc.sync.dma_start(mask_sb[:], mask_d[:])
            nc.sync.dma_start(psel_sb[:], psel_d[:])

            # current-layer tile handles (set by enqueue_weights)
            cur = {}

            def enqueue_weights(l, first):
                """Allocate layer-l weight tiles and enqueue all their DMAs
                on the SP ring. Order within the layer: biases, (feats),
                wpre groups 0-4, recurrence weights, wpre groups 5-12."""
                st = {}
                st["bf"] = w_p.tile([128, 28], F32, tag="bf", name="bf_sb")
                st["bb"] = w_p.tile([128, 24], F32, tag="bb", name="bb_sb")
                nc.sync.dma_start(st["bf"][:], biasf_d[l][:])
                nc.sync.dma_start(st["bb"][:], biasb_d[l][:])
                st["ft"] = pre_p.tile([128, 8, NCOL], B16, tag="ft", name="ftile")
                if first:
                    nc.sync.dma_start(
                        st["ft"][:].rearrange("p k c -> p (k c)"), featsT_d[:])
                st["wp"] = {}
                st["wf"] = w_p.tile([128, 8 * 24 * 128], F8, tag="wf", name="wf_sb")
                st["wb"] = w_p.tile([128, 4 * 20 * 128], F8, tag="wb", name="wb_sb")
                # PRE_B groups (7..12) stream first: their consumer (the bwd
                # root step) is early in the interleaved recurrence, so the
                # scheduler keeps their matmuls early and the next layer's
                # group DMAs (buffer-WAR on these readers) can prefetch.
                for i, gidx in enumerate(GORDER):
                    wpb = ws_p.tile([128, 8 * 4 * 128], B16, tag="wpre", name="wpb")
                    nc.sync.dma_start(wpb[:], wpre_d[l][gidx])
                    st["wp"][gidx] = wpb
                    if i == 4:
                        nc.sync.dma_start(st["wf"][:], wrecf_d[l][:])
                        nc.sync.dma_start(st["wb"][:], wrecb_d[l][:])
                return st

            def fwd_elem(lo, n, ps, lc, rc):
                """gates -> (c, hf) for fwd columns [lo, lo+n).
                gate order: ig og fl fr r u  (sigmoid 0:20, tanh 20:24).
                Gate pre-activations are carried x GSCALE; the activation
                scale undoes it exactly."""
                g = sc_p.tile([128, 24, 65], F32, tag="gates", name="g")
                if ps is None:
                    nc.scalar.activation(g[:, 0:20, :n], PRE_F[:, 0:20, lo:lo + n],
                                         AF.Sigmoid, scale=GINV)
                    nc.scalar.activation(g[:, 20:24, :n], PRE_F[:, 20:24, lo:lo + n],
                                         AF.Tanh, scale=GINV)
                else:
                    nc.vector.tensor_add(g[:, :, :n], ps[:, 0:24, :n],
                                         PRE_F[:, 0:24, lo:lo + n])
                    nc.scalar.activation(g[:, 0:20, :n], g[:, 0:20, :n], AF.Sigmoid,
                                         scale=GINV)
                    nc.scalar.activation(g[:, 20:24, :n], g[:, 20:24, :n], AF.Tanh,
                                         scale=GINV)
                cnew = CF[:, :, lo:lo + n]
                t1 = sc_p.tile([128, 4, 65], F32, tag="t1", name="t1")
                t2 = sc_p.tile([128, 4, 65], F32, tag="t2", name="t2")
                # c = ig*u (+ fl*lc + fr*rc)
                nc.vector.tensor_mul(cnew, g[:, 0:4, :n], g[:, 20:24, :n])
                if lc is not None:
                    nc.vector.tensor_mul(t1[:, :, :n], g[:, 8:12, :n], lc)
                    nc.vector.tensor_add(cnew, cnew, t1[:, :, :n])
                    nc.vector.tensor_mul(t2[:, :, :n], g[:, 12:16, :n], rc)
                    nc.vector.tensor_add(cnew, cnew, t2[:, :, :n])
                # hf = og*tanh(c)*r + (1-r)*px = r*(hh - px) + px
                nc.scalar.activation(t1[:, :, :n], cnew, AF.Tanh)
                nc.vector.tensor_mul(t2[:, :, :n], g[:, 4:8, :n], t1[:, :, :n])  # hh
                px = PRE_F[:, 24:28, lo:lo + n]
                nc.vector.tensor_sub(t2[:, :, :n], t2[:, :, :n], px)
                nc.vector.tensor_mul(t2[:, :, :n], g[:, 16:20, :n], t2[:, :, :n])
                nc.vector.tensor_add(HF[:, :, lo:lo + n], t2[:, :, :n], px)

            def bwd_elem(lo, n, ps, pc):
                # gate order: ig og f r u  (sigmoid 0:16, tanh 16:20)
                g = sc_p.tile([128, 24, 65], F32, tag="gates", name="gb")
                if ps is None:
                    nc.scalar.activation(g[:, 0:16, :n], PRE_B[:, 0:16, lo:lo + n],
                                         AF.Sigmoid, scale=GINV)
                    nc.scalar.activation(g[:, 16:20, :n], PRE_B[:, 16:20, lo:lo + n],
                                         AF.Tanh, scale=GINV)
                else:
                    nc.vector.tensor_add(g[:, 0:20, :n], ps[:, 0:20, :n],
                                         PRE_B[:, 0:20, lo:lo + n])
                    nc.scalar.activation(g[:, 0:16, :n], g[:, 0:16, :n], AF.Sigmoid,
                                         scale=GINV)
                    nc.scalar.activation(g[:, 16:20, :n], g[:, 16:20, :n], AF.Tanh,
                                         scale=GINV)
                cnew = CB[:, :, lo:lo + n]
                t1 = sc_p.tile([128, 4, 65], F32, tag="t1", name="t1b")
                t2 = sc_p.tile([128, 4, 65], F32, tag="t2", name="t2b")
                nc.vector.tensor_mul(cnew, g[:, 0:4, :n], g[:, 16:20, :n])  # ig*u
                if pc is not None:
                    nc.vector.tensor_mul(t1[:, :, :n], g[:, 8:12, :n], pc)
                    nc.vector.tensor_add(cnew, cnew, t1[:, :, :n])
                nc.scalar.activation(t1[:, :, :n], cnew, AF.Tanh)
                nc.vector.tensor_mul(t2[:, :, :n], g[:, 4:8, :n], t1[:, :, :n])
                px = PRE_B[:, 20:24, lo:lo + n]
                nc.vector.tensor_sub(t2[:, :, :n], t2[:, :, :n], px)
                nc.vector.tensor_mul(t2[:, :, :n], g[:, 12:16, :n], t2[:, :, :n])
                nc.vector.tensor_add(HB[:, :, lo:lo + n], t2[:, :, :n], px)

            def fwd_gemm_step(lo, n, clo, wf_ovr=None):
                ch = sc_p.tile([128, 8, 65], F8, tag="ch", name="ch")
                lc = sc_p.tile([128, 4, 65], F32, tag="lc", name="lc")
                rc = sc_p.tile([128, 4, 65], F32, tag="rc", name="rc")
                nc.vector.tensor_scalar_mul(ch[:, 0:4, :n],
                                            HF[:, :, clo:clo + 2 * n - 1:2],
                                            HSCALE)
                nc.vector.tensor_copy(lc[:, :, :n], CF[:, :, clo:clo + 2 * n - 1:2])
                nc.vector.tensor_scalar_mul(ch[:, 4:8, :n],
                                            HF[:, :, clo + 1:clo + 2 * n:2],
                                            HSCALE)
                nc.vector.tensor_copy(rc[:, :, :n], CF[:, :, clo + 1:clo + 2 * n:2])
                ps = pr_p.tile([128, 24, 64], F32, tag="rps", name="ps")
                wf_sb = wf_ovr if wf_ovr is not None else cur["wf"]
                for m in range(24):
                    for k in range(8):
                        nc.tensor.matmul(ps[:, m, :n],
                                         wf_sb[:, (k * 24 + m) * 128:(k * 24 + m + 1) * 128],
                                         ch[:, k, :n],
                                         start=(k == 0), stop=(k == 7))
                fwd_elem(lo, n, ps, lc[:, :, :n], rc[:, :, :n])

            def bwd_gemm_step(lo, n, plo):
                ch = sc_p.tile([128, 8, 65], F8, tag="ch", name="chb")
                pc = sc_p.tile([128, 4, 65], F32, tag="lc", name="pcb")
                if n == 1:
                    nc.vector.tensor_scalar_mul(ch[:, 0:4, 0:1],
                                                HB[:, :, plo:plo + 1], HSCALE)
                    nc.vector.tensor_copy(pc[:, :, 0:1], CB[:, :, plo:plo + 1])
                else:
                    m2 = n // 2
                    src_h = HB[:, :, plo:plo + m2].unsqueeze(3).broadcast_to(
                        [128, 4, m2, 2])
                    src_c = CB[:, :, plo:plo + m2].unsqueeze(3).broadcast_to(
                        [128, 4, m2, 2])
                    nc.vector.tensor_scalar_mul(
                        ch[:, 0:4, 0:n].rearrange("p c (a b) -> p c a b", b=2),
                        src_h, HSCALE)
                    nc.vector.tensor_copy(
                        pc[:, :, 0:n].rearrange("p c (a b) -> p c a b", b=2), src_c)
                ps = pr_p.tile([128, 24, 64], F32, tag="rps", name="psb")
                wb_sb = cur["wb"]
                for m in range(20):
                    for k in range(4):
                        nc.tensor.matmul(ps[:, m, :n],
                                         wb_sb[:, (k * 20 + m) * 128:(k * 20 + m + 1) * 128],
                                         ch[:, k, :n],
                                         start=(k == 0), stop=(k == 3))
                bwd_elem(lo, n, ps, pc[:, :, :n])

            def consume_gather(ccout, wf_sb, when_ms):
                """Gather-out DMAs + fwd top levels. The wait-until hint
                pins everything collective-gated late in the scheduler's
                modeled timeline: the DMAs stay off the SP ring ahead of
                the next layer's weight stream, and the fwd-top matmuls
                land AFTER the pinned bwd tail on the in-order PE queue
                (otherwise the PE would stall on them during the real
                AllGather with runnable bwd work stuck behind)."""
                with tc.tile_wait_until(when_ms):
                    for chn in range(4):
                        nc.sync.dma_start(
                            HF[:, chn, 135:143],
                            ccout[:, chn * 128:(chn + 1) * 128].rearrange(
                                "g p -> p g"))
                        nc.sync.dma_start(
                            CF[:, chn, 135:143],
                            ccout[:, 512 + chn * 128:512 + (chn + 1) * 128].rearrange(
                                "g p -> p g"))
                    fwd_gemm_step(131, 4, 135, wf_ovr=wf_sb)   # top level 2
                    fwd_gemm_step(129, 2, 131, wf_ovr=wf_sb)   # top level 1
                    fwd_gemm_step(128, 1, 129, wf_ovr=wf_sb)   # root

            pending_gather = None
            cur = enqueue_weights(0, first=True)

            for l in range(L):
                bf_sb, bb_sb = cur["bf"], cur["bb"]
                ftile = cur["ft"]

                PRE_F = pre_p.tile([128, 28, NCOL], F32, tag="pref", name="PRE_F")
                PRE_B = pre_p.tile([128, 24, NCOL], F32, tag="preb", name="PRE_B")

                if pending_gather is not None:
                    consume_gather(*pending_gather)
                    pending_gather = None

                if l > 0:
                    for k in range(8):
                        src = HF if k < 4 else HB
                        nc.vector.tensor_copy(ftile[:, k, :], src[:, k % 4, :])

                # ---- pre-projections: PRE = W_pre @ feats (feature-major) ----
                # psum tiles alternate between the pre pool and (views of) the
                # rec pool for a 4-deep rotation; the PSUM->PRE moves
                # alternate between the Act and DVE engines.
                for gidx in GORDER:
                    wpb = cur["wp"][gidx]
                    for mi in range(4):
                        m = gidx * 4 + mi
                        if m % 2 == 0:
                            ps = pp_p.tile([128, 143], F32, tag="pps", name="pps")
                        else:
                            psr = pr_p.tile([128, 24, 64], F32, tag="rps",
                                            name="ppsr")
                            ps = psr[:].rearrange("p a b -> p (a b)")[:, 0:143]
                        for k in range(8):
                            nc.tensor.matmul(
                                ps if m % 2 else ps[:],
                                wpb[:, (k * 4 + mi) * 128:(k * 4 + mi + 1) * 128],
                                ftile[:, k, :],
                                start=(k == 0), stop=(k == 7))
                        dst = (PRE_F[:, m, :] if m < 28
                               else PRE_B[:, m - 28, :])
                        bias = (bf_sb[:, m:m + 1] if m < 28
                                else bb_sb[:, m - 28:m - 27])
                        src = ps if m % 2 else ps[:]
                        if mi % 2 == 0:
                            nc.scalar.activation(dst, src, AF.Identity, bias=bias)
                        else:
                            nc.vector.tensor_scalar_add(dst, src, bias)

                # next layer's weight stream enqueues BEFORE the recurrence's
                # collective-dependent DMAs hit the SP ring
                nxt = enqueue_weights(l + 1, first=False) if l + 1 < L else None

                # ---- recurrences ----
                # fwd chain is the critical path to the AllGather; bwd steps
                # are interleaved so the PE can fill each chain's elementwise
                # latency with the other chain's matmuls.
                fwd_elem(63, 65, None, None, None)  # leaves (slots 63..127)
                bwd_elem(128, 1, None, None)        # root node 0
                # node-511 fix: slot 63 <- left child col 127 (masked), using
                # only the W_l half of wf (k-chunks 0..3). For cores != 0 the
                # mask zeroes the child, making this an idempotent leaf
                # recompute. Must run before the level-8 step below, which
                # consumes slot 63.
                chx = sc_p.tile([128, 8, 65], F8, tag="ch", name="chx")
                lcx = sc_p.tile([128, 4, 65], F32, tag="lc", name="lcx")
                rcx = sc_p.tile([128, 4, 65], F32, tag="rc", name="rcx")
                nc.vector.tensor_scalar(chx[:, 0:4, 0:1], HF[:, :, 127:128],
                                        HSCALE, mask_sb[:, 0:1],
                                        mybir.AluOpType.mult,
                                        mybir.AluOpType.mult)
                nc.vector.tensor_copy(lcx[:, :, 0:1], CF[:, :, 127:128])
                nc.vector.tensor_scalar_mul(lcx[:, :, 0:1], lcx[:, :, 0:1],
                                            mask_sb[:, 0:1])
                nc.vector.memset(rcx[:, :, 0:1], 0.0)
                psx = pr_p.tile([128, 24, 64], F32, tag="rps", name="psx")
                for m in range(24):
                    for k in range(4):
                        nc.tensor.matmul(
                            psx[:, m, 0:1],
                            cur["wf"][:, (k * 24 + m) * 128:(k * 24 + m + 1) * 128],
                            chx[:, k, 0:1], start=(k == 0), stop=(k == 3))
                fwd_elem(63, 1, psx, lcx[:, :, 0:1], rcx[:, :, 0:1])
                fwd_gemm_step(31, 32, 63)
                bwd_gemm_step(129, 2, 128)
                fwd_gemm_step(15, 16, 31)
                bwd_gemm_step(131, 4, 129)
                fwd_gemm_step(7, 8, 15)
                bwd_gemm_step(135, 8, 131)
                fwd_gemm_step(3, 4, 7)
                # copy own root (col 135+c) into local slot 0, then keep
                # interleaving bwd levels between the fwd top-of-subtree steps
                tmp = sc_p.tile([128, 4, 8], F32, tag="pseltmp", name="pseltmp")
                pb = psel_sb[:, :].unsqueeze(1).broadcast_to([128, 4, 8])
                nc.vector.tensor_mul(tmp[:], HB[:, :, 135:143], pb)
                nc.vector.reduce_sum(HB[:, :, 0], tmp[:], mybir.AxisListType.X)
                tmp2 = sc_p.tile([128, 4, 8], F32, tag="pseltmp", name="pseltmp2")
                nc.vector.tensor_mul(tmp2[:], CB[:, :, 135:143], pb)
                nc.vector.reduce_sum(CB[:, :, 0], tmp2[:], mybir.AxisListType.X)
                bwd_gemm_step(1, 2, 0)
                fwd_gemm_step(1, 2, 3)
                bwd_gemm_step(3, 4, 1)
                fwd_gemm_step(0, 1, 1)

                # AllGather the 8 subtree roots' (h, c)
                ccin = dram_p.tile([1024], F32, tag="ccin", name="ccin")
                ccout = dram_p.tile([8, 1024], F32, tag="ccout", name="ccout",
                                    addr_space="Shared")
                with tc.tile_wait_until(0.2 + 0.5 * l):
                    nc.sync.dma_start(
                        ccin[0:512].rearrange("(c p) -> p c", c=4, p=128),
                        HF[:, :, 0])
                    nc.sync.dma_start(
                        ccin[512:1024].rearrange("(c p) -> p c", c=4, p=128),
                        CF[:, :, 0])
                nc.gpsimd.collective_compute(
                    "AllGather", mybir.AluOpType.bypass,
                    ins=[ccin.opt()], outs=[ccout.opt()],
                    replica_groups=[list(range(NCORES))])
                pending_gather = (ccout, cur["wf"], 0.3 + 0.5 * l)

                # bwd tail: pinned into the AllGather's latency window on
                # the modeled timeline so the scheduler does not front-pack
                # it before the fwd chain finishes (the fwd-top steps are
                # pinned after it by consume_gather)
                with tc.tile_wait_until(0.25 + 0.5 * l):
                    bwd_gemm_step(7, 8, 3)
                    bwd_gemm_step(15, 16, 7)
                    bwd_gemm_step(31, 32, 15)
                    bwd_gemm_step(63, 64, 31)
                    bwd_gemm_step(127, 1, 63)    # node 1023

                if nxt is not None:
                    cur = nxt

            consume_gather(*pending_gather)

            # ---- outputs ----
            olv = out_loc_d[:].rearrange("(c p) n -> p c n", c=8, p=128)
            nc.sync.dma_start(olv[:, 0:4, :], HF[:, :, 0:128])
            nc.sync.dma_start(olv[:, 4:8, :], HB[:, :, 0:128])
            otv = out_top_d[:].rearrange("(c p) n -> p c n", c=8, p=128)
            nc.sync.dma_start(otv[:, 0:4, :], HF[:, :, 128:135])
            nc.sync.dma_start(otv[:, 4:8, :], HB[:, :, 128:135])

    nc.finalize()
    return nc


_program_cache = None


def kernel(features, f_px_w, f_px_b, f_x_w, f_x_b, f_l_w, f_l_b, f_r_w, f_r_b,
           b_px_w, b_px_b, b_x_w, b_x_b, b_h_w, b_h_b, left, right, parent):
    global _program_cache, _last_results
    features = np.asarray(features, dtype=np.float32)
    as32 = lambda a: np.asarray(a, dtype=np.float32)

    # ---- host-side packing (DRAM layout == SBUF layout, contiguous DMA) ----
    shared = {}
    for l in range(L):
        # gate rows of the pre-projections carry the x GSCALE fold (the px
        # highway rows stay raw)
        wpre = np.concatenate([_perm_f(as32(f_x_w[l])) * GSCALE, as32(f_px_w[l]),
                               _perm_b(as32(b_x_w[l])) * GSCALE, as32(b_px_w[l])],
                              axis=0)                    # [6656, 1024]
        t = _pack_lhsT(wpre, 8, 52, BF16)                # [8k, 52m, 128p, 128c]
        t = t.reshape(8, 13, 4, 128, 128).transpose(1, 3, 0, 2, 4)
        shared[f"wpre{l}"] = np.ascontiguousarray(t.reshape(13, 128, 4096))
        wrf = _perm_f(np.concatenate([as32(f_l_w[l]), as32(f_r_w[l])], axis=1))
        t = _pack_lhsT(wrf * WSCALE, 8, 24, FP8)         # [8, 24, 128, 128]
        shared[f"wrecf{l}"] = np.ascontiguousarray(
            t.transpose(2, 0, 1, 3).reshape(128, 8 * 24 * 128))
        t = _pack_lhsT(_perm_b(as32(b_h_w[l])) * WSCALE, 4, 20, FP8)
        shared[f"wrecb{l}"] = np.ascontiguousarray(
            t.transpose(2, 0, 1, 3).reshape(128, 4 * 20 * 128))
        bf = np.concatenate([_perm_f(as32(f_x_b[l]) + as32(f_l_b[l])
                                     + as32(f_r_b[l])) * GSCALE,
                             as32(f_px_b[l])])           # [3584]
        shared[f"biasf{l}"] = np.ascontiguousarray(bf.reshape(28, 128).T)
        bb = np.concatenate([_perm_b(as32(b_x_b[l]) + as32(b_h_b[l])) * GSCALE,
                             as32(b_px_b[l])])
        shared[f"biasb{l}"] = np.ascontiguousarray(bb.reshape(24, 128).T)

    in_maps = []
    ids_all = []
    for c in range(NCORES):
        ids = _node_ids(c)
        ids_all.append(ids)
        ft = features[ids].T.astype(BF16)                # [1024, 143]
        m = {k: v for k, v in shared.items()}
        m["featsT"] = np.ascontiguousarray(
            ft.reshape(8, 128, NCOL).transpose(1, 0, 2).reshape(128, 8 * NCOL))
        m["mask"] = np.full((128, 1), 1.0 if c == 0 else 0.0, np.float32)
        ps = np.zeros((128, 8), np.float32)
        ps[:, c] = 1.0
        m["psel"] = ps
        in_maps.append(m)

    if _program_cache is None:
        _program_cache = _build_program()
    nc = _program_cache

    trace = bool(os.environ.get("KERNEL_TRACE"))
    tdir = os.environ.get("KERNEL_TRACE_DIR") or None
    res = run_bass_kernel_spmd(nc, in_maps, core_ids=list(range(NCORES)),
                               trace=trace, tmpdir=tdir)
    _last_results = res

    out = np.empty((N, 2 * H), np.float32)
    for c in range(NCORES):
        loc = res.results[c]["out_loc"]                  # [1024, 128]
        nloc = 128 if c == 0 else 127
        out[ids_all[c][0:nloc]] = loc[:, 0:nloc].T
    out[0:7] = res.results[0]["out_top"].T
    return out


# revision 34
# speedup vs baseline: 1.0401x; 1.0401x over previous
"""Multi-layer bidirectional Tree-LSTM on 8 TRN2 NeuronCores.

Strategy: the input is a complete binary tree of 1024 nodes. Below level 3
there are 8 independent subtrees (rooted at nodes 7..14) -> one subtree per
core (data parallel). The top 7 nodes (0..6) are computed replicated on all
cores; one small AllGather per layer exchanges the 8 subtree-root (h, c)
pairs for the leaves->root direction.

On-device layout is feature-major (hidden dim on partitions, nodes on the
free axis); weights are stationary and node columns stream, so no
transposes are needed anywhere in the recurrence.

Per-core column layout (143 columns):
  0..126   : BFS slots of subtree(7+c)  (slot s, level k=floor(log2(s+1)))
  127      : node 1023 (replicated on every core; only core 0's is used)
  128..134 : top nodes 0..6 (replicated)
  135..142 : subtree roots 7..14 (fwd: from AllGather; bwd: replicated)

Precision: pre-projection weights bf16; recurrence weights fp8 e4m3
scaled by 64 (the recurrence is LDWEIGHTS-bandwidth-bound and fp8 FWL
loads 2x faster than bf16; the 1/64 un-scale is folded into the bf16
child/parent-h copies, which is a lossless exponent shift). Measured
end-to-end rel err ~6e-3 vs the 2e-2 gate.

Gate row order is permuted at pack time so all sigmoid gates are
contiguous: fwd [ig og fl fr r u] (sigmoid 0:20, tanh 20:24), bwd
[ig og f r u] (sigmoid 0:16, tanh 16:20).

All weight DRAM tensors are host-packed to the exact SBUF layout so every
weight DMA is a plain contiguous [128, X] copy (large descriptors, full
HBM bandwidth). The layer pipeline enqueues layer l+1's full weight
stream on the SP DMA ring BEFORE layer l's recurrence emits its
collective-dependent DMAs, so the next layer's weights prefetch during
the recurrence instead of queueing behind the AllGather.
"""

import os
import sys

for _p in ("/opt/trn_rl_repo",):
    if _p not in sys.path and os.path.isdir(_p):
        sys.path.insert(0, _p)

import numpy as np
import ml_dtypes

try:
    import jax
    jax.config.update("jax_compilation_cache_dir", os.environ.get("KERNEL_JAX_CACHE", "/tmp/jax_neff_cache"))
    jax.config.update("jax_persistent_cache_min_compile_time_secs", 5.0)
    jax.config.update("jax_persistent_cache_min_entry_size_bytes", 0)
except Exception:
    pass

import concourse.bass as bass
import concourse.mybir as mybir
from concourse import bacc
from concourse.tile import TileContext
from concourse.bass_utils import run_bass_kernel_spmd

BF16 = ml_dtypes.bfloat16
FP8 = ml_dtypes.float8_e4m3fn
F32 = mybir.dt.float32
B16 = mybir.dt.bfloat16
F8 = mybir.dt.float8e4
AF = mybir.ActivationFunctionType

N, D, H, L = 1024, 1024, 512, 2
NCOL = 143  # 127 subtree + node1023 + 7 top + 8 roots
NCORES = 8
# fp8 recurrence scales: weights x64, h operand x4. The product (x256) is
# folded into the pre-projection gate rows/biases on the host, and undone
# by the activation `scale` parameter (powers of two -> exact).
WSCALE = 64.0
HSCALE = 4.0
GSCALE = WSCALE * HSCALE
GINV = 1.0 / GSCALE
# pre-projection group streaming order: PRE_B groups first
GORDER = [7, 8, 9, 10, 11, 12, 0, 1, 2, 3, 4, 5, 6]

_last_results = None  # stashed BassKernelResults for test.py


def _node_ids(c):
    ids = []
    for k in range(7):
        base = (8 + c) * (1 << k) - 1
        ids.extend(range(base, base + (1 << k)))
    ids.append(1023)
    ids.extend(range(0, 7))
    ids.extend(range(7, 15))
    return np.asarray(ids, dtype=np.int64)


def _pack_lhsT(w, kchunks, mchunks, dty):
    # w: [M, K] fp32 -> lhsT tiles [kchunks, mchunks, 128, 128] where
    # tile[k, m, kp, mc] = w[m*128+mc, k*128+kp]
    Mdim, Kdim = w.shape
    assert Mdim == mchunks * 128 and Kdim == kchunks * 128
    t = w.reshape(mchunks, 128, kchunks, 128).transpose(2, 0, 3, 1)
    return np.ascontiguousarray(t.astype(dty))


def _perm_f(w):
    # fwd gate rows [ig og fl fr u r] -> [ig og fl fr r u]
    return np.concatenate([w[0:2048], w[2560:3072], w[2048:2560]], axis=0)


def _perm_b(w):
    # bwd gate rows [ig og f u r] -> [ig og f r u]
    return np.concatenate([w[0:1536], w[2048:2560], w[1536:2048]], axis=0)


def _build_program():
    nc = bacc.Bacc("TRN2", target_bir_lowering=False, debug=False,
                   num_devices=NCORES)

    featsT_d = nc.dram_tensor("featsT", [128, 8 * NCOL], B16, kind="ExternalInput")
    wpre_d, wrecf_d, wrecb_d, biasf_d, biasb_d = [], [], [], [], []
    for l in range(L):
        wpre_d.append(nc.dram_tensor(f"wpre{l}", [13, 128, 4096], B16,
                                     kind="ExternalInput"))
        wrecf_d.append(nc.dram_tensor(f"wrecf{l}", [128, 8 * 24 * 128], F8,
                                      kind="ExternalInput"))
        wrecb_d.append(nc.dram_tensor(f"wrecb{l}", [128, 4 * 20 * 128], F8,
                                      kind="ExternalInput"))
        biasf_d.append(nc.dram_tensor(f"biasf{l}", [128, 28], F32,
                                      kind="ExternalInput"))
        biasb_d.append(nc.dram_tensor(f"biasb{l}", [128, 24], F32,
                                      kind="ExternalInput"))
    mask_d = nc.dram_tensor("mask", [128, 1], F32, kind="ExternalInput")
    psel_d = nc.dram_tensor("psel", [128, 8], F32, kind="ExternalInput")
    out_loc_d = nc.dram_tensor("out_loc", [1024, 128], F32, kind="ExternalOutput")
    out_top_d = nc.dram_tensor("out_top", [1024, 7], F32, kind="ExternalOutput")

    with TileContext(nc) as tc:
        with (
            tc.tile_pool(name="state", bufs=1) as state_p,
            tc.tile_pool(name="weights", bufs=2) as w_p,
            tc.tile_pool(name="pre", bufs=1) as pre_p,
            tc.tile_pool(name="wstream", bufs=9) as ws_p,
            tc.tile_pool(name="scratch", bufs=2) as sc_p,
            tc.tile_pool(name="psum_pre", bufs=2, space="PSUM") as pp_p,
            tc.tile_pool(name="psum_rec", bufs=2, space="PSUM") as pr_p,
            tc.tile_pool(name="dram", bufs=1, space="DRAM") as dram_p,
        ):
            HF = state_p.tile([128, 4, NCOL], F32, name="HF")
            CF = state_p.tile([128, 4, NCOL], F32, name="CF")
            HB = state_p.tile([128, 4, NCOL], F32, name="HB")
            CB = state_p.tile([128, 4, NCOL], F32, name="CB")
            mask_sb = state_p.tile([128, 1], F32, name="mask_sb")
            psel_sb = state_p.tile([128, 8], F32, name="psel_sb")
            nc.sync.dma_start(mask_sb[:], mask_d[:])
            nc.sync.dma_start(psel_sb[:], psel_d[:])

            # current-layer tile handles (set by enqueue_weights)
            cur = {}

            def enqueue_weights(l, first):
                """Allocate layer-l weight tiles and enqueue all their DMAs
                on the SP ring. Order within the layer: biases, (feats),
                wpre groups 0-4, recurrence weights, wpre groups 5-12."""
                st = {}
                st["bf"] = w_p.tile([128, 28], F32, tag="bf", name="bf_sb")
                st["bb"] = w_p.tile([128, 24], F32, tag="bb", name="bb_sb")
                nc.sync.dma_start(st["bf"][:], biasf_d[l][:])
                nc.sync.dma_start(st["bb"][:], biasb_d[l][:])
                st["ft"] = pre_p.tile([128, 8, NCOL], B16, tag="ft", name="ftile")
                if first:
                    nc.sync.dma_start(
                        st["ft"][:].rearrange("p k c -> p (k c)"), featsT_d[:])
                st["wp"] = {}
                st["wf"] = w_p.tile([128, 8 * 24 * 128], F8, tag="wf", name="wf_sb")
                st["wb"] = w_p.tile([128, 4 * 20 * 128], F8, tag="wb", name="wb_sb")
                # PRE_B groups (7..12) stream first: their consumer (the bwd
                # root step) is early in the interleaved recurrence, so the
                # scheduler keeps their matmuls early and the next layer's
                # group DMAs (buffer-WAR on these readers) can prefetch.
                for i, gidx in enumerate(GORDER):
                    wpb = ws_p.tile([128, 8 * 4 * 128], B16, tag="wpre", name="wpb")
                    nc.sync.dma_start(wpb[:], wpre_d[l][gidx])
                    st["wp"][gidx] = wpb
                    if i == 4:
                        nc.sync.dma_start(st["wf"][:], wrecf_d[l][:])
                        nc.sync.dma_start(st["wb"][:], wrecb_d[l][:])
                return st

            def fwd_elem(lo, n, ps, lc, rc):
                """gates -> (c, hf) for fwd columns [lo, lo+n).
                gate order: ig og fl fr r u  (sigmoid 0:20, tanh 20:24).
                Gate pre-activations are carried x GSCALE; the activation
                scale undoes it exactly."""
                g = sc_p.tile([128, 24, 65], F32, tag="gates", name="g")
                if ps is None:
                    nc.scalar.activation(g[:, 0:20, :n], PRE_F[:, 0:20, lo:lo + n],
                                         AF.Sigmoid, scale=GINV)
                    nc.scalar.activation(g[:, 20:24, :n], PRE_F[:, 20:24, lo:lo + n],
                                         AF.Tanh, scale=GINV)
                else:
                    nc.vector.tensor_add(g[:, :, :n], ps[:, 0:24, :n],
                                         PRE_F[:, 0:24, lo:lo + n])
                    nc.scalar.activation(g[:, 0:20, :n], g[:, 0:20, :n], AF.Sigmoid,
                                         scale=GINV)
                    nc.scalar.activation(g[:, 20:24, :n], g[:, 20:24, :n], AF.Tanh,
                                         scale=GINV)
                cnew = CF[:, :, lo:lo + n]
                t1 = sc_p.tile([128, 4, 65], F32, tag="t1", name="t1")
                t2 = sc_p.tile([128, 4, 65], F32, tag="t2", name="t2")
                # c = ig*u (+ fl*lc + fr*rc)
                nc.vector.tensor_mul(cnew, g[:, 0:4, :n], g[:, 20:24, :n])
                if lc is not None:
                    nc.vector.tensor_mul(t1[:, :, :n], g[:, 8:12, :n], lc)
                    nc.vector.tensor_add(cnew, cnew, t1[:, :, :n])
                    nc.vector.tensor_mul(t2[:, :, :n], g[:, 12:16, :n], rc)
                    nc.vector.tensor_add(cnew, cnew, t2[:, :, :n])
                # hf = og*tanh(c)*r + (1-r)*px = r*(hh - px) + px
                nc.scalar.activation(t1[:, :, :n], cnew, AF.Tanh)
                nc.vector.tensor_mul(t2[:, :, :n], g[:, 4:8, :n], t1[:, :, :n])  # hh
                px = PRE_F[:, 24:28, lo:lo + n]
                nc.vector.tensor_sub(t2[:, :, :n], t2[:, :, :n], px)
                nc.vector.tensor_mul(t2[:, :, :n], g[:, 16:20, :n], t2[:, :, :n])
                nc.vector.tensor_add(HF[:, :, lo:lo + n], t2[:, :, :n], px)

            def bwd_elem(lo, n, ps, pc):
                # gate order: ig og f r u  (sigmoid 0:16, tanh 16:20)
                g = sc_p.tile([128, 24, 65], F32, tag="gates", name="gb")
                if ps is None:
                    nc.scalar.activation(g[:, 0:16, :n], PRE_B[:, 0:16, lo:lo + n],
                                         AF.Sigmoid, scale=GINV)
                    nc.scalar.activation(g[:, 16:20, :n], PRE_B[:, 16:20, lo:lo + n],
                                         AF.Tanh, scale=GINV)
                else:
                    nc.vector.tensor_add(g[:, 0:20, :n], ps[:, 0:20, :n],
                                         PRE_B[:, 0:20, lo:lo + n])
                    nc.scalar.activation(g[:, 0:16, :n], g[:, 0:16, :n], AF.Sigmoid,
                                         scale=GINV)
                    nc.scalar.activation(g[:, 16:20, :n], g[:, 16:20, :n], AF.Tanh,
                                         scale=GINV)
                cnew = CB[:, :, lo:lo + n]
                t1 = sc_p.tile([128, 4, 65], F32, tag="t1", name="t1b")
                t2 = sc_p.tile([128, 4, 65], F32, tag="t2", name="t2b")
                nc.vector.tensor_mul(cnew, g[:, 0:4, :n], g[:, 16:20, :n])  # ig*u
                if pc is not None:
                    nc.vector.tensor_mul(t1[:, :, :n], g[:, 8:12, :n], pc)
                    nc.vector.tensor_add(cnew, cnew, t1[:, :, :n])
                nc.scalar.activation(t1[:, :, :n], cnew, AF.Tanh)
                nc.vector.tensor_mul(t2[:, :, :n], g[:, 4:8, :n], t1[:, :, :n])
                px = PRE_B[:, 20:24, lo:lo + n]
                nc.vector.tensor_sub(t2[:, :, :n], t2[:, :, :n], px)
                nc.vector.tensor_mul(t2[:, :, :n], g[:, 12:16, :n], t2[:, :, :n])
                nc.vector.tensor_add(HB[:, :, lo:lo + n], t2[:, :, :n], px)

            def fwd_gemm_step(lo, n, clo, wf_ovr=None):
                ch = sc_p.tile([128, 8, 65], F8, tag="ch", name="ch")
                lc = sc_p.tile([128, 4, 65], F32, tag="lc", name="lc")
                rc = sc_p.tile([128, 4, 65], F32, tag="rc", name="rc")
                nc.vector.tensor_scalar_mul(ch[:, 0:4, :n],
                                            HF[:, :, clo:clo + 2 * n - 1:2],
                                            HSCALE)
                nc.vector.tensor_copy(lc[:, :, :n], CF[:, :, clo:clo + 2 * n - 1:2])
                nc.vector.tensor_scalar_mul(ch[:, 4:8, :n],
                                            HF[:, :, clo + 1:clo + 2 * n:2],
                                            HSCALE)
                nc.vector.tensor_copy(rc[:, :, :n], CF[:, :, clo + 1:clo + 2 * n:2])
                ps = pr_p.tile([128, 24, 64], F32, tag="rps", name="ps")
                wf_sb = wf_ovr if wf_ovr is not None else cur["wf"]
                for m in range(24):
                    for k in range(8):
                        nc.tensor.matmul(ps[:, m, :n],
                                         wf_sb[:, (k * 24 + m) * 128:(k * 24 + m + 1) * 128],
                                         ch[:, k, :n],
                                         start=(k == 0), stop=(k == 7))
                fwd_elem(lo, n, ps, lc[:, :, :n], rc[:, :, :n])

            def bwd_gemm_step(lo, n, plo, after=None):
                ch = sc_p.tile([128, 8, 65], F8, tag="ch", name="chb")
                pc = sc_p.tile([128, 4, 65], F32, tag="lc", name="pcb")
                if after is not None:
                    # dependency injection: a throwaway write into ch that
                    # reads `after` holds this step (and the chain behind
                    # it) until `after` is produced — both in the
                    # scheduler's model and on hardware. Keeps the bwd tail
                    # inside the AllGather's latency window instead of
                    # being front-packed before the fwd chain ends.
                    nc.vector.tensor_scalar_mul(ch[:, 0:1, 0:1], after, HSCALE)
                if n == 1:
                    nc.vector.tensor_scalar_mul(ch[:, 0:4, 0:1],
                                                HB[:, :, plo:plo + 1], HSCALE)
                    nc.vector.tensor_copy(pc[:, :, 0:1], CB[:, :, plo:plo + 1])
                else:
                    m2 = n // 2
                    src_h = HB[:, :, plo:plo + m2].unsqueeze(3).broadcast_to(
                        [128, 4, m2, 2])
                    src_c = CB[:, :, plo:plo + m2].unsqueeze(3).broadcast_to(
                        [128, 4, m2, 2])
                    nc.vector.tensor_scalar_mul(
                        ch[:, 0:4, 0:n].rearrange("p c (a b) -> p c a b", b=2),
                        src_h, HSCALE)
                    nc.vector.tensor_copy(
                        pc[:, :, 0:n].rearrange("p c (a b) -> p c a b", b=2), src_c)
                ps = pr_p.tile([128, 24, 64], F32, tag="rps", name="psb")
                wb_sb = cur["wb"]
                for m in range(20):
                    for k in range(4):
                        nc.tensor.matmul(ps[:, m, :n],
                                         wb_sb[:, (k * 20 + m) * 128:(k * 20 + m + 1) * 128],
                                         ch[:, k, :n],
                                         start=(k == 0), stop=(k == 3))
                bwd_elem(lo, n, ps, pc[:, :, :n])

            def consume_gather(ccout, wf_sb):
                """Gather-out DMAs + fwd top levels (gated on the collective
                by their data dependencies)."""
                for chn in range(4):
                    nc.sync.dma_start(
                        HF[:, chn, 135:143],
                        ccout[:, chn * 128:(chn + 1) * 128].rearrange(
                            "g p -> p g"))
                    nc.sync.dma_start(
                        CF[:, chn, 135:143],
                        ccout[:, 512 + chn * 128:512 + (chn + 1) * 128].rearrange(
                            "g p -> p g"))
                fwd_gemm_step(131, 4, 135, wf_ovr=wf_sb)   # top level 2
                fwd_gemm_step(129, 2, 131, wf_ovr=wf_sb)   # top level 1
                fwd_gemm_step(128, 1, 129, wf_ovr=wf_sb)   # root

            pending_gather = None
            cur = enqueue_weights(0, first=True)

            for l in range(L):
                bf_sb, bb_sb = cur["bf"], cur["bb"]
                ftile = cur["ft"]

                PRE_F = pre_p.tile([128, 28, NCOL], F32, tag="pref", name="PRE_F")
                PRE_B = pre_p.tile([128, 24, NCOL], F32, tag="preb", name="PRE_B")

                if pending_gather is not None:
                    consume_gather(*pending_gather)
                    pending_gather = None

                if l > 0:
                    for k in range(8):
                        src = HF if k < 4 else HB
                        nc.vector.tensor_copy(ftile[:, k, :], src[:, k % 4, :])

                # ---- pre-projections: PRE = W_pre @ feats (feature-major) ----
                # psum tiles alternate between the pre pool and (views of) the
                # rec pool for a 4-deep rotation; the PSUM->PRE moves
                # alternate between the Act and DVE engines.
                for gidx in GORDER:
                    wpb = cur["wp"][gidx]
                    for mi in range(4):
                        m = gidx * 4 + mi
                        if m % 2 == 0:
                            ps = pp_p.tile([128, 143], F32, tag="pps", name="pps")
                        else:
                            psr = pr_p.tile([128, 24, 64], F32, tag="rps",
                                            name="ppsr")
                            ps = psr[:].rearrange("p a b -> p (a b)")[:, 0:143]
                        for k in range(8):
                            nc.tensor.matmul(
                                ps if m % 2 else ps[:],
                                wpb[:, (k * 4 + mi) * 128:(k * 4 + mi + 1) * 128],
                                ftile[:, k, :],
                                start=(k == 0), stop=(k == 7))
                        dst = (PRE_F[:, m, :] if m < 28
                               else PRE_B[:, m - 28, :])
                        bias = (bf_sb[:, m:m + 1] if m < 28
                                else bb_sb[:, m - 28:m - 27])
                        src = ps if m % 2 else ps[:]
                        if mi % 2 == 0:
                            nc.scalar.activation(dst, src, AF.Identity, bias=bias)
                        else:
                            nc.vector.tensor_scalar_add(dst, src, bias)

                # next layer's weight stream enqueues BEFORE the recurrence's
                # collective-dependent DMAs hit the SP ring
                nxt = enqueue_weights(l + 1, first=False) if l + 1 < L else None

                # ---- recurrences ----
                # fwd chain is the critical path to the AllGather; bwd steps
                # are interleaved so the PE can fill each chain's elementwise
                # latency with the other chain's matmuls.
                fwd_elem(63, 65, None, None, None)  # leaves (slots 63..127)
                bwd_elem(128, 1, None, None)        # root node 0
                # node-511 fix: slot 63 <- left child col 127 (masked), using
                # only the W_l half of wf (k-chunks 0..3). For cores != 0 the
                # mask zeroes the child, making this an idempotent leaf
                # recompute. Must run before the level-8 step below, which
                # consumes slot 63.
                chx = sc_p.tile([128, 8, 65], F8, tag="ch", name="chx")
                lcx = sc_p.tile([128, 4, 65], F32, tag="lc", name="lcx")
                rcx = sc_p.tile([128, 4, 65], F32, tag="rc", name="rcx")
                nc.vector.tensor_scalar(chx[:, 0:4, 0:1], HF[:, :, 127:128],
                                        HSCALE, mask_sb[:, 0:1],
                                        mybir.AluOpType.mult,
                                        mybir.AluOpType.mult)
                nc.vector.tensor_copy(lcx[:, :, 0:1], CF[:, :, 127:128])
                nc.vector.tensor_scalar_mul(lcx[:, :, 0:1], lcx[:, :, 0:1],
                                            mask_sb[:, 0:1])
                nc.vector.memset(rcx[:, :, 0:1], 0.0)
                psx = pr_p.tile([128, 24, 64], F32, tag="rps", name="psx")
                for m in range(24):
                    for k in range(4):
                        nc.tensor.matmul(
                            psx[:, m, 0:1],
                            cur["wf"][:, (k * 24 + m) * 128:(k * 24 + m + 1) * 128],
                            chx[:, k, 0:1], start=(k == 0), stop=(k == 3))
                fwd_elem(63, 1, psx, lcx[:, :, 0:1], rcx[:, :, 0:1])
                fwd_gemm_step(31, 32, 63)
                bwd_gemm_step(129, 2, 128)
                fwd_gemm_step(15, 16, 31)
                bwd_gemm_step(131, 4, 129)
                fwd_gemm_step(7, 8, 15)
                bwd_gemm_step(135, 8, 131)
                fwd_gemm_step(3, 4, 7)
                # copy own root (col 135+c) into local slot 0, then keep
                # interleaving bwd levels between the fwd top-of-subtree steps
                tmp = sc_p.tile([128, 4, 8], F32, tag="pseltmp", name="pseltmp")
                pb = psel_sb[:, :].unsqueeze(1).broadcast_to([128, 4, 8])
                nc.vector.tensor_mul(tmp[:], HB[:, :, 135:143], pb)
                nc.vector.reduce_sum(HB[:, :, 0], tmp[:], mybir.AxisListType.X)
                tmp2 = sc_p.tile([128, 4, 8], F32, tag="pseltmp", name="pseltmp2")
                nc.vector.tensor_mul(tmp2[:], CB[:, :, 135:143], pb)
                nc.vector.reduce_sum(CB[:, :, 0], tmp2[:], mybir.AxisListType.X)
                bwd_gemm_step(1, 2, 0)
                fwd_gemm_step(1, 2, 3)
                bwd_gemm_step(3, 4, 1)
                fwd_gemm_step(0, 1, 1)

                # AllGather the 8 subtree roots' (h, c)
                ccin = dram_p.tile([1024], F32, tag="ccin", name="ccin")
                ccout = dram_p.tile([8, 1024], F32, tag="ccout", name="ccout",
                                    addr_space="Shared")
                nc.sync.dma_start(
                    ccin[0:512].rearrange("(c p) -> p c", c=4, p=128),
                    HF[:, :, 0])
                nc.sync.dma_start(
                    ccin[512:1024].rearrange("(c p) -> p c", c=4, p=128),
                    CF[:, :, 0])
                nc.gpsimd.collective_compute(
                    "AllGather", mybir.AluOpType.bypass,
                    ins=[ccin.opt()], outs=[ccout.opt()],
                    replica_groups=[list(range(NCORES))])
                pending_gather = (ccout, cur["wf"])

                # bwd tail: held until the fwd root is produced (dependency
                # injection) so it fills the AllGather's latency window on
                # the PE instead of being front-packed earlier
                bwd_gemm_step(7, 8, 3, after=HF[:, 0:1, 0:1])
                bwd_gemm_step(15, 16, 7)
                bwd_gemm_step(31, 32, 15)
                bwd_gemm_step(63, 64, 31)
                bwd_gemm_step(127, 1, 63)    # node 1023

                if nxt is not None:
                    cur = nxt

            consume_gather(*pending_gather)

            # ---- outputs ----
            olv = out_loc_d[:].rearrange("(c p) n -> p c n", c=8, p=128)
            nc.sync.dma_start(olv[:, 0:4, :], HF[:, :, 0:128])
            nc.sync.dma_start(olv[:, 4:8, :], HB[:, :, 0:128])
            otv = out_top_d[:].rearrange("(c p) n -> p c n", c=8, p=128)
            nc.sync.dma_start(otv[:, 0:4, :], HF[:, :, 128:135])
            nc.sync.dma_start(otv[:, 4:8, :], HB[:, :, 128:135])

    nc.finalize()
    return nc


_program_cache = None


def kernel(features, f_px_w, f_px_b, f_x_w, f_x_b, f_l_w, f_l_b, f_r_w, f_r_b,
           b_px_w, b_px_b, b_x_w, b_x_b, b_h_w, b_h_b, left, right, parent):
    global _program_cache, _last_results
    features = np.asarray(features, dtype=np.float32)
    as32 = lambda a: np.asarray(a, dtype=np.float32)

    # ---- host-side packing (DRAM layout == SBUF layout, contiguous DMA) ----
    shared = {}
    for l in range(L):
        # gate rows of the pre-projections carry the x GSCALE fold (the px
        # highway rows stay raw)
        wpre = np.concatenate([_perm_f(as32(f_x_w[l])) * GSCALE, as32(f_px_w[l]),
                               _perm_b(as32(b_x_w[l])) * GSCALE, as32(b_px_w[l])],
                              axis=0)                    # [6656, 1024]
        t = _pack_lhsT(wpre, 8, 52, BF16)                # [8k, 52m, 128p, 128c]
        t = t.reshape(8, 13, 4, 128, 128).transpose(1, 3, 0, 2, 4)
        shared[f"wpre{l}"] = np.ascontiguousarray(t.reshape(13, 128, 4096))
        wrf = _perm_f(np.concatenate([as32(f_l_w[l]), as32(f_r_w[l])], axis=1))
        t = _pack_lhsT(wrf * WSCALE, 8, 24, FP8)         # [8, 24, 128, 128]
        shared[f"wrecf{l}"] = np.ascontiguousarray(
            t.transpose(2, 0, 1, 3).reshape(128, 8 * 24 * 128))
        t = _pack_lhsT(_perm_b(as32(b_h_w[l])) * WSCALE, 4, 20, FP8)
        shared[f"wrecb{l}"] = np.ascontiguousarray(
            t.transpose(2, 0, 1, 3).reshape(128, 4 * 20 * 128))
        bf = np.concatenate([_perm_f(as32(f_x_b[l]) + as32(f_l_b[l])
                                     + as32(f_r_b[l])) * GSCALE,
                             as32(f_px_b[l])])           # [3584]
        shared[f"biasf{l}"] = np.ascontiguousarray(bf.reshape(28, 128).T)
        bb = np.concatenate([_perm_b(as32(b_x_b[l]) + as32(b_h_b[l])) * GSCALE,
                             as32(b_px_b[l])])
        shared[f"biasb{l}"] = np.ascontiguousarray(bb.reshape(24, 128).T)

    in_maps = []
    ids_all = []
    for c in range(NCORES):
        ids = _node_ids(c)
        ids_all.append(ids)
        ft = features[ids].T.astype(BF16)                # [1024, 143]
        m = {k: v for k, v in shared.items()}
        m["featsT"] = np.ascontiguousarray(
            ft.reshape(8, 128, NCOL).transpose(1, 0, 2).reshape(128, 8 * NCOL))
        m["mask"] = np.full((128, 1), 1.0 if c == 0 else 0.0, np.float32)
        ps = np.zeros((128, 8), np.float32)
        ps[:, c] = 1.0
        m["psel"] = ps
        in_maps.append(m)

    if _program_cache is None:
        _program_cache = _build_program()
    nc = _program_cache

    trace = bool(os.environ.get("KERNEL_TRACE"))
    tdir = os.environ.get("KERNEL_TRACE_DIR") or None
    res = run_bass_kernel_spmd(nc, in_maps, core_ids=list(range(NCORES)),
                               trace=trace, tmpdir=tdir)
    _last_results = res

    out = np.empty((N, 2 * H), np.float32)
    for c in range(NCORES):
        loc = res.results[c]["out_loc"]                  # [1024, 128]
        nloc = 128 if c == 0 else 127
        out[ids_all[c][0:nloc]] = loc[:, 0:nloc].T
    out[0:7] = res.results[0]["out_top"].T
    return out


# revision 41
# speedup vs baseline: 1.0915x; 1.0495x over previous
"""Multi-layer bidirectional Tree-LSTM on 8 TRN2 NeuronCores.

Strategy: the input is a complete binary tree of 1024 nodes. Below level 3
there are 8 independent subtrees (rooted at nodes 7..14) -> one subtree per
core (data parallel). The top 7 nodes (0..6) are computed replicated on all
cores; one small AllGather per layer exchanges the 8 subtree-root (h, c)
pairs for the leaves->root direction.

On-device layout is feature-major (hidden dim on partitions, nodes on the
free axis); weights are stationary and node columns stream, so no
transposes are needed anywhere in the recurrence.

Per-core column layout (143 columns):
  0..126   : BFS slots of subtree(7+c)  (slot s, level k=floor(log2(s+1)))
  127      : node 1023 (replicated on every core; only core 0's is used)
  128..134 : top nodes 0..6 (replicated)
  135..142 : subtree roots 7..14 (fwd: from AllGather; bwd: replicated)

Precision: pre-projection weights bf16; recurrence weights fp8 e4m3
scaled by 64 (the recurrence is LDWEIGHTS-bandwidth-bound and fp8 FWL
loads 2x faster than bf16; the 1/64 un-scale is folded into the bf16
child/parent-h copies, which is a lossless exponent shift). Measured
end-to-end rel err ~6e-3 vs the 2e-2 gate.

Gate row order is permuted at pack time so all sigmoid gates are
contiguous: fwd [ig og fl fr r u] (sigmoid 0:20, tanh 20:24), bwd
[ig og f r u] (sigmoid 0:16, tanh 16:20).

All weight DRAM tensors are host-packed to the exact SBUF layout so every
weight DMA is a plain contiguous [128, X] copy (large descriptors, full
HBM bandwidth). The layer pipeline enqueues layer l+1's full weight
stream on the SP DMA ring BEFORE layer l's recurrence emits its
collective-dependent DMAs, so the next layer's weights prefetch during
the recurrence instead of queueing behind the AllGather.
"""

import os
import sys

for _p in ("/opt/trn_rl_repo",):
    if _p not in sys.path and os.path.isdir(_p):
        sys.path.insert(0, _p)

import numpy as np
import ml_dtypes

try:
    import jax
    jax.config.update("jax_compilation_cache_dir", os.environ.get("KERNEL_JAX_CACHE", "/tmp/jax_neff_cache"))
    jax.config.update("jax_persistent_cache_min_compile_time_secs", 5.0)
    jax.config.update("jax_persistent_cache_min_entry_size_bytes", 0)
except Exception:
    pass

import concourse.bass as bass
import concourse.mybir as mybir
from concourse import bacc
from concourse.tile import TileContext
from concourse.bass_utils import run_bass_kernel_spmd

BF16 = ml_dtypes.bfloat16
FP8 = ml_dtypes.float8_e4m3fn
F32 = mybir.dt.float32
B16 = mybir.dt.bfloat16
F8 = mybir.dt.float8e4
AF = mybir.ActivationFunctionType

N, D, H, L = 1024, 1024, 512, 2
NCOL = 143  # 127 subtree + node1023 + 7 top + 8 roots
NCORES = 8
# fp8 recurrence scales: weights x64, h operand x4. The product (x256) is
# folded into the pre-projection gate rows/biases on the host, and undone
# by the activation `scale` parameter (powers of two -> exact).
WSCALE = 64.0
HSCALE = 4.0
GSCALE = WSCALE * HSCALE
GINV = 1.0 / GSCALE
# pre-projection group streaming order
GORDER = [0, 1, 2, 3, 4, 5, 6, 7, 8, 9, 10, 11, 12]

_last_results = None  # stashed BassKernelResults for test.py


def _node_ids(c):
    ids = []
    for k in range(7):
        base = (8 + c) * (1 << k) - 1
        ids.extend(range(base, base + (1 << k)))
    ids.append(1023)
    ids.extend(range(0, 7))
    ids.extend(range(7, 15))
    return np.asarray(ids, dtype=np.int64)


def _pack_lhsT(w, kchunks, mchunks, dty):
    # w: [M, K] fp32 -> lhsT tiles [kchunks, mchunks, 128, 128] where
    # tile[k, m, kp, mc] = w[m*128+mc, k*128+kp]
    Mdim, Kdim = w.shape
    assert Mdim == mchunks * 128 and Kdim == kchunks * 128
    t = w.reshape(mchunks, 128, kchunks, 128).transpose(2, 0, 3, 1)
    return np.ascontiguousarray(t.astype(dty))


def _perm_f(w):
    # fwd gate rows [ig og fl fr u r] -> [ig og fl fr r u]
    return np.concatenate([w[0:2048], w[2560:3072], w[2048:2560]], axis=0)


def _perm_b(w):
    # bwd gate rows [ig og f u r] -> [ig og f r u]
    return np.concatenate([w[0:1536], w[2048:2560], w[1536:2048]], axis=0)


def _build_program():
    nc = bacc.Bacc("TRN2", target_bir_lowering=False, debug=False,
                   num_devices=NCORES)

    featsT_d = nc.dram_tensor("featsT", [128, 8 * NCOL], B16, kind="ExternalInput")
    wpre_d, wrecf_d, wrecb_d, biasf_d, biasb_d = [], [], [], [], []
    for l in range(L):
        wpre_d.append(nc.dram_tensor(f"wpre{l}", [13, 128, 4096], B16,
                                     kind="ExternalInput"))
        wrecf_d.append(nc.dram_tensor(f"wrecf{l}", [128, 8 * 24 * 128], F8,
                                      kind="ExternalInput"))
        wrecb_d.append(nc.dram_tensor(f"wrecb{l}", [128, 4 * 20 * 128], F8,
                                      kind="ExternalInput"))
        biasf_d.append(nc.dram_tensor(f"biasf{l}", [128, 28], F32,
                                      kind="ExternalInput"))
        biasb_d.append(nc.dram_tensor(f"biasb{l}", [128, 24], F32,
                                      kind="ExternalInput"))
    mask_d = nc.dram_tensor("mask", [128, 1], F32, kind="ExternalInput")
    psel_d = nc.dram_tensor("psel", [128, 8], F32, kind="ExternalInput")
    out_loc_d = nc.dram_tensor("out_loc", [1024, 128], F32, kind="ExternalOutput")
    out_top_d = nc.dram_tensor("out_top", [1024, 7], F32, kind="ExternalOutput")

    with TileContext(nc) as tc:
        with (
            tc.tile_pool(name="state", bufs=1) as state_p,
            tc.tile_pool(name="weights", bufs=2) as w_p,
            tc.tile_pool(name="pre", bufs=1) as pre_p,
            tc.tile_pool(name="wstream", bufs=8) as ws_p,
            tc.tile_pool(name="scratch", bufs=2) as sc_p,
            tc.tile_pool(name="psum_pre", bufs=2, space="PSUM") as pp_p,
            tc.tile_pool(name="psum_rec", bufs=2, space="PSUM") as pr_p,
            tc.tile_pool(name="dram", bufs=1, space="DRAM") as dram_p,
        ):
            HF = state_p.tile([128, 4, NCOL], F32, name="HF")
            CF = state_p.tile([128, 4, NCOL], F32, name="CF")
            HB = state_p.tile([128, 4, NCOL], F32, name="HB")
            CB = state_p.tile([128, 4, NCOL], F32, name="CB")
            mask_sb = state_p.tile([128, 1], F32, name="mask_sb")
            psel_sb = state_p.tile([128, 8], F32, name="psel_sb")
            nc.sync.dma_start(mask_sb[:], mask_d[:])
            nc.sync.dma_start(psel_sb[:], psel_d[:])

            # current-layer tile handles (set by enqueue_weights)
            cur = {}

            def enqueue_weights(l, first):
                """Allocate layer-l weight tiles and enqueue all their DMAs
                on the SP ring. Order within the layer: biases, (feats),
                wpre groups 0-4, recurrence weights, wpre groups 5-12."""
                st = {}
                st["bf"] = w_p.tile([128, 28], F32, tag="bf", name="bf_sb")
                st["bb"] = w_p.tile([128, 24], F32, tag="bb", name="bb_sb")
                nc.sync.dma_start(st["bf"][:], biasf_d[l][:])
                nc.sync.dma_start(st["bb"][:], biasb_d[l][:])
                st["ft"] = pre_p.tile([128, 8, NCOL], B16, tag="ft", name="ftile")
                if first:
                    nc.sync.dma_start(
                        st["ft"][:].rearrange("p k c -> p (k c)"), featsT_d[:])
                st["wp"] = {}
                st["wf"] = w_p.tile([128, 8 * 24 * 128], F8, tag="wf", name="wf_sb")
                st["wb"] = w_p.tile([128, 4 * 20 * 128], F8, tag="wb", name="wb_sb")
                # PRE_B groups (7..12) stream first: their consumer (the bwd
                # root step) is early in the interleaved recurrence, so the
                # scheduler keeps their matmuls early and the next layer's
                # group DMAs (buffer-WAR on these readers) can prefetch.
                for i, gidx in enumerate(GORDER):
                    wpb = ws_p.tile([128, 8 * 4 * 128], B16, tag="wpre", name="wpb")
                    nc.sync.dma_start(wpb[:], wpre_d[l][gidx])
                    st["wp"][gidx] = wpb
                    if i == 4:
                        nc.sync.dma_start(st["wf"][:], wrecf_d[l][:])
                        nc.sync.dma_start(st["wb"][:], wrecb_d[l][:])
                return st

            def fwd_elem(lo, n, ps, lc, rc):
                """gates -> (c, hf) for fwd columns [lo, lo+n).
                gate order: ig og fl fr r u  (sigmoid 0:20, tanh 20:24).
                Gate pre-activations are carried x GSCALE; the activation
                scale undoes it exactly."""
                g = sc_p.tile([128, 24, 65], F32, tag="gates", name="g")
                if ps is None:
                    nc.scalar.activation(g[:, 0:20, :n], PRE_F[:, 0:20, lo:lo + n],
                                         AF.Sigmoid, scale=GINV)
                    nc.scalar.activation(g[:, 20:24, :n], PRE_F[:, 20:24, lo:lo + n],
                                         AF.Tanh, scale=GINV)
                else:
                    nc.vector.tensor_add(g[:, :, :n], ps[:, 0:24, :n],
                                         PRE_F[:, 0:24, lo:lo + n])
                    nc.scalar.activation(g[:, 0:20, :n], g[:, 0:20, :n], AF.Sigmoid,
                                         scale=GINV)
                    nc.scalar.activation(g[:, 20:24, :n], g[:, 20:24, :n], AF.Tanh,
                                         scale=GINV)
                cnew = CF[:, :, lo:lo + n]
                t1 = sc_p.tile([128, 4, 65], F32, tag="t1", name="t1")
                t2 = sc_p.tile([128, 4, 65], F32, tag="t2", name="t2")
                # c = ig*u (+ fl*lc + fr*rc)
                nc.vector.tensor_mul(cnew, g[:, 0:4, :n], g[:, 20:24, :n])
                if lc is not None:
                    nc.vector.tensor_mul(t1[:, :, :n], g[:, 8:12, :n], lc)
                    nc.vector.tensor_add(cnew, cnew, t1[:, :, :n])
                    nc.vector.tensor_mul(t2[:, :, :n], g[:, 12:16, :n], rc)
                    nc.vector.tensor_add(cnew, cnew, t2[:, :, :n])
                # hf = og*tanh(c)*r + (1-r)*px = r*(hh - px) + px
                nc.scalar.activation(t1[:, :, :n], cnew, AF.Tanh)
                nc.vector.tensor_mul(t2[:, :, :n], g[:, 4:8, :n], t1[:, :, :n])  # hh
                px = PRE_F[:, 24:28, lo:lo + n]
                nc.vector.tensor_sub(t2[:, :, :n], t2[:, :, :n], px)
                nc.vector.tensor_mul(t2[:, :, :n], g[:, 16:20, :n], t2[:, :, :n])
                nc.vector.tensor_add(HF[:, :, lo:lo + n], t2[:, :, :n], px)

            def bwd_elem(lo, n, ps, pc):
                # gate order: ig og f r u  (sigmoid 0:16, tanh 16:20)
                g = sc_p.tile([128, 24, 65], F32, tag="gates", name="gb")
                if ps is None:
                    nc.scalar.activation(g[:, 0:16, :n], PRE_B[:, 0:16, lo:lo + n],
                                         AF.Sigmoid, scale=GINV)
                    nc.scalar.activation(g[:, 16:20, :n], PRE_B[:, 16:20, lo:lo + n],
                                         AF.Tanh, scale=GINV)
                else:
                    nc.vector.tensor_add(g[:, 0:20, :n], ps[:, 0:20, :n],
                                         PRE_B[:, 0:20, lo:lo + n])
                    nc.scalar.activation(g[:, 0:16, :n], g[:, 0:16, :n], AF.Sigmoid,
                                         scale=GINV)
                    nc.scalar.activation(g[:, 16:20, :n], g[:, 16:20, :n], AF.Tanh,
                                         scale=GINV)
                cnew = CB[:, :, lo:lo + n]
                t1 = sc_p.tile([128, 4, 65], F32, tag="t1", name="t1b")
                t2 = sc_p.tile([128, 4, 65], F32, tag="t2", name="t2b")
                nc.vector.tensor_mul(cnew, g[:, 0:4, :n], g[:, 16:20, :n])  # ig*u
                if pc is not None:
                    nc.vector.tensor_mul(t1[:, :, :n], g[:, 8:12, :n], pc)
                    nc.vector.tensor_add(cnew, cnew, t1[:, :, :n])
                nc.scalar.activation(t1[:, :, :n], cnew, AF.Tanh)
                nc.vector.tensor_mul(t2[:, :, :n], g[:, 4:8, :n], t1[:, :, :n])
                px = PRE_B[:, 20:24, lo:lo + n]
                nc.vector.tensor_sub(t2[:, :, :n], t2[:, :, :n], px)
                nc.vector.tensor_mul(t2[:, :, :n], g[:, 12:16, :n], t2[:, :, :n])
                nc.vector.tensor_add(HB[:, :, lo:lo + n], t2[:, :, :n], px)

            def fwd_gemm_step(lo, n, clo, wf_ovr=None):
                ch = sc_p.tile([128, 8, 65], F8, tag="ch", name="ch")
                lc = sc_p.tile([128, 4, 65], F32, tag="lc", name="lc")
                rc = sc_p.tile([128, 4, 65], F32, tag="rc", name="rc")
                nc.vector.tensor_scalar_mul(ch[:, 0:4, :n],
                                            HF[:, :, clo:clo + 2 * n - 1:2],
                                            HSCALE)
                nc.vector.tensor_copy(lc[:, :, :n], CF[:, :, clo:clo + 2 * n - 1:2])
                nc.vector.tensor_scalar_mul(ch[:, 4:8, :n],
                                            HF[:, :, clo + 1:clo + 2 * n:2],
                                            HSCALE)
                nc.vector.tensor_copy(rc[:, :, :n], CF[:, :, clo + 1:clo + 2 * n:2])
                ps = pr_p.tile([128, 24, 64], F32, tag="rps", name="ps")
                wf_sb = wf_ovr if wf_ovr is not None else cur["wf"]
                for m in range(24):
                    for k in range(8):
                        nc.tensor.matmul(ps[:, m, :n],
                                         wf_sb[:, (k * 24 + m) * 128:(k * 24 + m + 1) * 128],
                                         ch[:, k, :n],
                                         start=(k == 0), stop=(k == 7))
                fwd_elem(lo, n, ps, lc[:, :, :n], rc[:, :, :n])

            def bwd_gemm_step(lo, n, plo, after=None):
                ch = sc_p.tile([128, 8, 65], F8, tag="ch", name="chb")
                pc = sc_p.tile([128, 4, 65], F32, tag="lc", name="pcb")
                if after is not None:
                    # dependency injection: a throwaway write into ch that
                    # reads `after` holds this step (and the chain behind
                    # it) until `after` is produced — both in the
                    # scheduler's model and on hardware. Keeps the bwd tail
                    # inside the AllGather's latency window instead of
                    # being front-packed before the fwd chain ends.
                    nc.vector.tensor_scalar_mul(ch[:, 0:1, 0:1], after, HSCALE)
                if n == 1:
                    nc.vector.tensor_scalar_mul(ch[:, 0:4, 0:1],
                                                HB[:, :, plo:plo + 1], HSCALE)
                    nc.vector.tensor_copy(pc[:, :, 0:1], CB[:, :, plo:plo + 1])
                else:
                    m2 = n // 2
                    src_h = HB[:, :, plo:plo + m2].unsqueeze(3).broadcast_to(
                        [128, 4, m2, 2])
                    src_c = CB[:, :, plo:plo + m2].unsqueeze(3).broadcast_to(
                        [128, 4, m2, 2])
                    nc.vector.tensor_scalar_mul(
                        ch[:, 0:4, 0:n].rearrange("p c (a b) -> p c a b", b=2),
                        src_h, HSCALE)
                    nc.vector.tensor_copy(
                        pc[:, :, 0:n].rearrange("p c (a b) -> p c a b", b=2), src_c)
                ps = pr_p.tile([128, 24, 64], F32, tag="rps", name="psb")
                wb_sb = cur["wb"]
                for m in range(20):
                    for k in range(4):
                        nc.tensor.matmul(ps[:, m, :n],
                                         wb_sb[:, (k * 20 + m) * 128:(k * 20 + m + 1) * 128],
                                         ch[:, k, :n],
                                         start=(k == 0), stop=(k == 3))
                bwd_elem(lo, n, ps, pc[:, :, :n])

            def consume_gather(ccout, wf_sb, when_ms):
                """Gather-out DMAs + fwd top levels. The wait-until hint
                keeps the collective-gated DMAs from occupying SP-ring
                slots ahead of the next layer's weight stream."""
                with tc.tile_wait_until(when_ms):
                    for chn in range(4):
                        nc.sync.dma_start(
                            HF[:, chn, 135:143],
                            ccout[:, chn * 128:(chn + 1) * 128].rearrange(
                                "g p -> p g"))
                        nc.sync.dma_start(
                            CF[:, chn, 135:143],
                            ccout[:, 512 + chn * 128:512 + (chn + 1) * 128].rearrange(
                                "g p -> p g"))
                fwd_gemm_step(131, 4, 135, wf_ovr=wf_sb)   # top level 2
                fwd_gemm_step(129, 2, 131, wf_ovr=wf_sb)   # top level 1
                fwd_gemm_step(128, 1, 129, wf_ovr=wf_sb)   # root

            pending_gather = None
            cur = enqueue_weights(0, first=True)

            for l in range(L):
                bf_sb, bb_sb = cur["bf"], cur["bb"]
                ftile = cur["ft"]

                PRE_F = pre_p.tile([128, 28, NCOL], F32, tag="pref", name="PRE_F")
                PRE_B = pre_p.tile([128, 24, NCOL], F32, tag="preb", name="PRE_B")

                if pending_gather is not None:
                    consume_gather(*pending_gather)
                    pending_gather = None

                if l > 0:
                    for k in range(8):
                        src = HF if k < 4 else HB
                        nc.vector.tensor_copy(ftile[:, k, :], src[:, k % 4, :])

                # ---- pre-projections: PRE = W_pre @ feats (feature-major) ----
                # the PSUM->PRE moves alternate between the Act and DVE
                # engines so the 2-buffer psum rotation is reader-limited
                # by neither engine alone.
                for gidx in GORDER:
                    wpb = cur["wp"][gidx]
                    for mi in range(4):
                        m = gidx * 4 + mi
                        ps = pp_p.tile([128, 143], F32, tag="pps", name="pps")
                        for k in range(8):
                            nc.tensor.matmul(
                                ps[:],
                                wpb[:, (k * 4 + mi) * 128:(k * 4 + mi + 1) * 128],
                                ftile[:, k, :],
                                start=(k == 0), stop=(k == 7))
                        dst = (PRE_F[:, m, :] if m < 28
                               else PRE_B[:, m - 28, :])
                        bias = (bf_sb[:, m:m + 1] if m < 28
                                else bb_sb[:, m - 28:m - 27])
                        if mi % 2 == 0:
                            nc.scalar.activation(dst, ps[:], AF.Identity,
                                                 bias=bias)
                        else:
                            nc.vector.tensor_scalar_add(dst, ps[:], bias)

                # next layer's weight stream enqueues BEFORE the recurrence's
                # collective-dependent DMAs hit the SP ring
                nxt = enqueue_weights(l + 1, first=False) if l + 1 < L else None

                # ---- recurrences ----
                # fwd chain is the critical path to the AllGather; bwd steps
                # are interleaved so the PE can fill each chain's elementwise
                # latency with the other chain's matmuls.
                fwd_elem(63, 65, None, None, None)  # leaves (slots 63..127)
                bwd_elem(128, 1, None, None)        # root node 0
                # node-511 fix: slot 63 <- left child col 127 (masked), using
                # only the W_l half of wf (k-chunks 0..3). For cores != 0 the
                # mask zeroes the child, making this an idempotent leaf
                # recompute. Must run before the level-8 step below, which
                # consumes slot 63.
                chx = sc_p.tile([128, 8, 65], F8, tag="ch", name="chx")
                lcx = sc_p.tile([128, 4, 65], F32, tag="lc", name="lcx")
                rcx = sc_p.tile([128, 4, 65], F32, tag="rc", name="rcx")
                nc.vector.tensor_scalar(chx[:, 0:4, 0:1], HF[:, :, 127:128],
                                        HSCALE, mask_sb[:, 0:1],
                                        mybir.AluOpType.mult,
                                        mybir.AluOpType.mult)
                nc.vector.tensor_copy(lcx[:, :, 0:1], CF[:, :, 127:128])
                nc.vector.tensor_scalar_mul(lcx[:, :, 0:1], lcx[:, :, 0:1],
                                            mask_sb[:, 0:1])
                nc.vector.memset(rcx[:, :, 0:1], 0.0)
                psx = pr_p.tile([128, 24, 64], F32, tag="rps", name="psx")
                for m in range(24):
                    for k in range(4):
                        nc.tensor.matmul(
                            psx[:, m, 0:1],
                            cur["wf"][:, (k * 24 + m) * 128:(k * 24 + m + 1) * 128],
                            chx[:, k, 0:1], start=(k == 0), stop=(k == 3))
                fwd_elem(63, 1, psx, lcx[:, :, 0:1], rcx[:, :, 0:1])
                fwd_gemm_step(31, 32, 63)
                bwd_gemm_step(129, 2, 128)
                fwd_gemm_step(15, 16, 31)
                bwd_gemm_step(131, 4, 129)
                fwd_gemm_step(7, 8, 15)
                bwd_gemm_step(135, 8, 131)
                fwd_gemm_step(3, 4, 7)
                fwd_gemm_step(1, 2, 3)
                fwd_gemm_step(0, 1, 1)

                # AllGather the 8 subtree roots' (h, c)
                ccin = dram_p.tile([1024], F32, tag="ccin", name="ccin")
                ccout = dram_p.tile([8, 1024], F32, tag="ccout", name="ccout",
                                    addr_space="Shared")
                with tc.tile_wait_until(0.04 + 0.2 * l):
                    nc.sync.dma_start(
                        ccin[0:512].rearrange("(c p) -> p c", c=4, p=128),
                        HF[:, :, 0])
                    nc.sync.dma_start(
                        ccin[512:1024].rearrange("(c p) -> p c", c=4, p=128),
                        CF[:, :, 0])
                nc.gpsimd.collective_compute(
                    "AllGather", mybir.AluOpType.bypass,
                    ins=[ccin.opt()], outs=[ccout.opt()],
                    replica_groups=[list(range(NCORES))])
                pending_gather = (ccout, cur["wf"], 0.05 + 0.2 * l)

                # rest of bwd chain (independent of the AllGather)
                # copy own root (col 135+c) into local slot 0
                tmp = sc_p.tile([128, 4, 8], F32, tag="pseltmp", name="pseltmp")
                pb = psel_sb[:, :].unsqueeze(1).broadcast_to([128, 4, 8])
                nc.vector.tensor_mul(tmp[:], HB[:, :, 135:143], pb)
                nc.vector.reduce_sum(HB[:, :, 0], tmp[:], mybir.AxisListType.X)
                tmp2 = sc_p.tile([128, 4, 8], F32, tag="pseltmp", name="pseltmp2")
                nc.vector.tensor_mul(tmp2[:], CB[:, :, 135:143], pb)
                nc.vector.reduce_sum(CB[:, :, 0], tmp2[:], mybir.AxisListType.X)
                bwd_gemm_step(1, 2, 0)
                bwd_gemm_step(3, 4, 1)
                bwd_gemm_step(7, 8, 3)
                bwd_gemm_step(15, 16, 7)
                bwd_gemm_step(31, 32, 15)
                bwd_gemm_step(63, 64, 31)
                bwd_gemm_step(127, 1, 63)    # node 1023

                if nxt is not None:
                    cur = nxt

            consume_gather(*pending_gather)

            # ---- outputs ----
            olv = out_loc_d[:].rearrange("(c p) n -> p c n", c=8, p=128)
            nc.sync.dma_start(olv[:, 0:4, :], HF[:, :, 0:128])
            nc.sync.dma_start(olv[:, 4:8, :], HB[:, :, 0:128])
            otv = out_top_d[:].rearrange("(c p) n -> p c n", c=8, p=128)
            nc.sync.dma_start(otv[:, 0:4, :], HF[:, :, 128:135])
            nc.sync.dma_start(otv[:, 4:8, :], HB[:, :, 128:135])

    nc.finalize()
    return nc


_program_cache = None


def kernel(features, f_px_w, f_px_b, f_x_w, f_x_b, f_l_w, f_l_b, f_r_w, f_r_b,
           b_px_w, b_px_b, b_x_w, b_x_b, b_h_w, b_h_b, left, right, parent):
    global _program_cache, _last_results
    features = np.asarray(features, dtype=np.float32)
    as32 = lambda a: np.asarray(a, dtype=np.float32)

    # ---- host-side packing (DRAM layout == SBUF layout, contiguous DMA) ----
    shared = {}
    for l in range(L):
        # gate rows of the pre-projections carry the x GSCALE fold (the px
        # highway rows stay raw)
        wpre = np.concatenate([_perm_f(as32(f_x_w[l])) * GSCALE, as32(f_px_w[l]),
                               _perm_b(as32(b_x_w[l])) * GSCALE, as32(b_px_w[l])],
                              axis=0)                    # [6656, 1024]
        t = _pack_lhsT(wpre, 8, 52, BF16)                # [8k, 52m, 128p, 128c]
        t = t.reshape(8, 13, 4, 128, 128).transpose(1, 3, 0, 2, 4)
        shared[f"wpre{l}"] = np.ascontiguousarray(t.reshape(13, 128, 4096))
        wrf = _perm_f(np.concatenate([as32(f_l_w[l]), as32(f_r_w[l])], axis=1))
        t = _pack_lhsT(wrf * WSCALE, 8, 24, FP8)         # [8, 24, 128, 128]
        shared[f"wrecf{l}"] = np.ascontiguousarray(
            t.transpose(2, 0, 1, 3).reshape(128, 8 * 24 * 128))
        t = _pack_lhsT(_perm_b(as32(b_h_w[l])) * WSCALE, 4, 20, FP8)
        shared[f"wrecb{l}"] = np.ascontiguousarray(
            t.transpose(2, 0, 1, 3).reshape(128, 4 * 20 * 128))
        bf = np.concatenate([_perm_f(as32(f_x_b[l]) + as32(f_l_b[l])
                                     + as32(f_r_b[l])) * GSCALE,
                             as32(f_px_b[l])])           # [3584]
        shared[f"biasf{l}"] = np.ascontiguousarray(bf.reshape(28, 128).T)
        bb = np.concatenate([_perm_b(as32(b_x_b[l]) + as32(b_h_b[l])) * GSCALE,
                             as32(b_px_b[l])])
        shared[f"biasb{l}"] = np.ascontiguousarray(bb.reshape(24, 128).T)

    in_maps = []
    ids_all = []
    for c in range(NCORES):
        ids = _node_ids(c)
        ids_all.append(ids)
        ft = features[ids].T.astype(BF16)                # [1024, 143]
        m = {k: v for k, v in shared.items()}
        m["featsT"] = np.ascontiguousarray(
            ft.reshape(8, 128, NCOL).transpose(1, 0, 2).reshape(128, 8 * NCOL))
        m["mask"] = np.full((128, 1), 1.0 if c == 0 else 0.0, np.float32)
        ps = np.zeros((128, 8), np.float32)
        ps[:, c] = 1.0
        m["psel"] = ps
        in_maps.append(m)

    if _program_cache is None:
        _program_cache = _build_program()
    nc = _program_cache

    trace = bool(os.environ.get("KERNEL_TRACE"))
    tdir = os.environ.get("KERNEL_TRACE_DIR") or None
    res = run_bass_kernel_spmd(nc, in_maps, core_ids=list(range(NCORES)),
                               trace=trace, tmpdir=tdir)
    _last_results = res

    out = np.empty((N, 2 * H), np.float32)
    for c in range(NCORES):
        loc = res.results[c]["out_loc"]                  # [1024, 128]
        nloc = 128 if c == 0 else 127
        out[ids_all[c][0:nloc]] = loc[:, 0:nloc].T
    out[0:7] = res.results[0]["out_top"].T
    return out


# revision 45
# speedup vs baseline: 1.1766x; 1.0780x over previous
"""Multi-layer bidirectional Tree-LSTM on 8 TRN2 NeuronCores.

Strategy: the input is a complete binary tree of 1024 nodes. Below level 3
there are 8 independent subtrees (rooted at nodes 7..14) -> one subtree per
core (data parallel). The top 7 nodes (0..6) are computed replicated on all
cores; one small AllGather per layer exchanges the 8 subtree-root (h, c)
pairs for the leaves->root direction.

On-device layout is feature-major (hidden dim on partitions, nodes on the
free axis); weights are stationary and node columns stream, so no
transposes are needed anywhere in the recurrence.

Per-core column layout (143 columns):
  0..126   : BFS slots of subtree(7+c)  (slot s, level k=floor(log2(s+1)))
  127      : node 1023 (replicated on every core; only core 0's is used)
  128..134 : top nodes 0..6 (replicated)
  135..142 : subtree roots 7..14 (fwd: from AllGather; bwd: replicated)

Precision: pre-projection weights bf16; recurrence weights fp8 e4m3
scaled by 64 (the recurrence is LDWEIGHTS-bandwidth-bound and fp8 FWL
loads 2x faster than bf16; the 1/64 un-scale is folded into the bf16
child/parent-h copies, which is a lossless exponent shift). Measured
end-to-end rel err ~6e-3 vs the 2e-2 gate.

Gate row order is permuted at pack time so all sigmoid gates are
contiguous: fwd [ig og fl fr r u] (sigmoid 0:20, tanh 20:24), bwd
[ig og f r u] (sigmoid 0:16, tanh 16:20).

All weight DRAM tensors are host-packed to the exact SBUF layout so every
weight DMA is a plain contiguous [128, X] copy (large descriptors, full
HBM bandwidth). The layer pipeline enqueues layer l+1's full weight
stream on the SP DMA ring BEFORE layer l's recurrence emits its
collective-dependent DMAs, so the next layer's weights prefetch during
the recurrence instead of queueing behind the AllGather.
"""

import os
import sys

for _p in ("/opt/trn_rl_repo",):
    if _p not in sys.path and os.path.isdir(_p):
        sys.path.insert(0, _p)

import numpy as np
import ml_dtypes

try:
    import jax
    jax.config.update("jax_compilation_cache_dir", os.environ.get("KERNEL_JAX_CACHE", "/tmp/jax_neff_cache"))
    jax.config.update("jax_persistent_cache_min_compile_time_secs", 5.0)
    jax.config.update("jax_persistent_cache_min_entry_size_bytes", 0)
except Exception:
    pass

import concourse.bass as bass
import concourse.mybir as mybir
from concourse import bacc
from concourse.tile import TileContext
from concourse.bass_utils import run_bass_kernel_spmd

BF16 = ml_dtypes.bfloat16
FP8 = ml_dtypes.float8_e4m3fn
F32 = mybir.dt.float32
B16 = mybir.dt.bfloat16
F8 = mybir.dt.float8e4
AF = mybir.ActivationFunctionType

N, D, H, L = 1024, 1024, 512, 2
NCOL = 143  # 127 subtree + node1023 + 7 top + 8 roots
NCORES = 8
# fp8 recurrence scales: weights x64, h operand x4. The product (x256) is
# folded into the pre-projection gate rows/biases on the host, and undone
# by the activation `scale` parameter (powers of two -> exact).
WSCALE = 64.0
HSCALE = 4.0
GSCALE = WSCALE * HSCALE
GINV = 1.0 / GSCALE
# pre-projection group streaming order: PRE_B groups (7-12) first, so their
# matmuls (whose consumer, the bwd root step, is early in the interleaved
# recurrence) are not deferred by the scheduler -- the next layer's group
# DMAs buffer-WAR on these readers
GORDER = [7, 8, 9, 10, 11, 12, 0, 1, 2, 3, 4, 5, 6]

_last_results = None  # stashed BassKernelResults for test.py


def _node_ids(c):
    ids = []
    for k in range(7):
        base = (8 + c) * (1 << k) - 1
        ids.extend(range(base, base + (1 << k)))
    ids.append(1023)
    ids.extend(range(0, 7))
    ids.extend(range(7, 15))
    return np.asarray(ids, dtype=np.int64)


def _pack_lhsT(w, kchunks, mchunks, dty):
    # w: [M, K] fp32 -> lhsT tiles [kchunks, mchunks, 128, 128] where
    # tile[k, m, kp, mc] = w[m*128+mc, k*128+kp]
    Mdim, Kdim = w.shape
    assert Mdim == mchunks * 128 and Kdim == kchunks * 128
    t = w.reshape(mchunks, 128, kchunks, 128).transpose(2, 0, 3, 1)
    return np.ascontiguousarray(t.astype(dty))


def _perm_f(w):
    # fwd gate rows [ig og fl fr u r] -> [ig og fl fr r u]
    return np.concatenate([w[0:2048], w[2560:3072], w[2048:2560]], axis=0)


def _perm_b(w):
    # bwd gate rows [ig og f u r] -> [ig og f r u]
    return np.concatenate([w[0:1536], w[2048:2560], w[1536:2048]], axis=0)


def _build_program():
    nc = bacc.Bacc("TRN2", target_bir_lowering=False, debug=False,
                   num_devices=NCORES)

    featsT_d = nc.dram_tensor("featsT", [128, 8 * NCOL], B16, kind="ExternalInput")
    wpre_d, wrecf_d, wrecb_d, biasf_d, biasb_d = [], [], [], [], []
    for l in range(L):
        wpre_d.append(nc.dram_tensor(f"wpre{l}", [13, 128, 4096], B16,
                                     kind="ExternalInput"))
        wrecf_d.append(nc.dram_tensor(f"wrecf{l}", [128, 8 * 24 * 128], F8,
                                      kind="ExternalInput"))
        wrecb_d.append(nc.dram_tensor(f"wrecb{l}", [128, 4 * 20 * 128], F8,
                                      kind="ExternalInput"))
        biasf_d.append(nc.dram_tensor(f"biasf{l}", [128, 28], F32,
                                      kind="ExternalInput"))
        biasb_d.append(nc.dram_tensor(f"biasb{l}", [128, 24], F32,
                                      kind="ExternalInput"))
    mask_d = nc.dram_tensor("mask", [128, 1], F32, kind="ExternalInput")
    psel_d = nc.dram_tensor("psel", [128, 8], F32, kind="ExternalInput")
    out_loc_d = nc.dram_tensor("out_loc", [1024, 128], F32, kind="ExternalOutput")
    out_top_d = nc.dram_tensor("out_top", [1024, 7], F32, kind="ExternalOutput")

    with TileContext(nc) as tc:
        with (
            tc.tile_pool(name="state", bufs=1) as state_p,
            tc.tile_pool(name="weights", bufs=2) as w_p,
            tc.tile_pool(name="pre", bufs=1) as pre_p,
            tc.tile_pool(name="wstream", bufs=8) as ws_p,
            tc.tile_pool(name="scratch", bufs=2) as sc_p,
            tc.tile_pool(name="psum_pre", bufs=2, space="PSUM") as pp_p,
            tc.tile_pool(name="psum_rec", bufs=2, space="PSUM") as pr_p,
            tc.tile_pool(name="dram", bufs=1, space="DRAM") as dram_p,
        ):
            HF = state_p.tile([128, 4, NCOL], F32, name="HF")
            CF = state_p.tile([128, 4, NCOL], F32, name="CF")
            HB = state_p.tile([128, 4, NCOL], F32, name="HB")
            CB = state_p.tile([128, 4, NCOL], F32, name="CB")
            mask_sb = state_p.tile([128, 1], F32, name="mask_sb")
            psel_sb = state_p.tile([128, 8], F32, name="psel_sb")
            nc.sync.dma_start(mask_sb[:], mask_d[:])
            nc.sync.dma_start(psel_sb[:], psel_d[:])

            # warmup collective: the first AllGather of a run costs ~3-5x
            # the steady-state latency. Absorb that during the initial
            # weight streaming (CC cores are idle there) with a tiny dummy
            # gather so the layer-0 AllGather runs warm.
            ccw_in = dram_p.tile([8], F32, tag="ccwi", name="ccw_in")
            ccw_out = dram_p.tile([8, 8], F32, tag="ccwo", name="ccw_out",
                                  addr_space="Shared")
            nc.sync.dma_start(ccw_in[0:8], psel_d[0, 0:8])
            nc.gpsimd.collective_compute(
                "AllGather", mybir.AluOpType.bypass,
                ins=[ccw_in.opt()], outs=[ccw_out.opt()],
                replica_groups=[list(range(NCORES))])

            # current-layer tile handles (set by enqueue_weights)
            cur = {}

            def enqueue_weights(l, first):
                """Allocate layer-l weight tiles and enqueue all their DMAs
                on the SP ring. Order within the layer: biases, (feats),
                wpre groups 0-4, recurrence weights, wpre groups 5-12."""
                st = {}
                st["bf"] = w_p.tile([128, 28], F32, tag="bf", name="bf_sb")
                st["bb"] = w_p.tile([128, 24], F32, tag="bb", name="bb_sb")
                nc.sync.dma_start(st["bf"][:], biasf_d[l][:])
                nc.sync.dma_start(st["bb"][:], biasb_d[l][:])
                st["ft"] = pre_p.tile([128, 8, NCOL], B16, tag="ft", name="ftile")
                if first:
                    nc.sync.dma_start(
                        st["ft"][:].rearrange("p k c -> p (k c)"), featsT_d[:])
                st["wp"] = {}
                st["wf"] = w_p.tile([128, 8 * 24 * 128], F8, tag="wf", name="wf_sb")
                st["wb"] = w_p.tile([128, 4 * 20 * 128], F8, tag="wb", name="wb_sb")
                # PRE_B groups (7..12) stream first: their consumer (the bwd
                # root step) is early in the interleaved recurrence, so the
                # scheduler keeps their matmuls early and the next layer's
                # group DMAs (buffer-WAR on these readers) can prefetch.
                for i, gidx in enumerate(GORDER):
                    wpb = ws_p.tile([128, 8 * 4 * 128], B16, tag="wpre", name="wpb")
                    nc.sync.dma_start(wpb[:], wpre_d[l][gidx])
                    st["wp"][gidx] = wpb
                    if i == 4:
                        nc.sync.dma_start(st["wf"][:], wrecf_d[l][:])
                        nc.sync.dma_start(st["wb"][:], wrecb_d[l][:])
                return st

            def fwd_elem(lo, n, ps, lc, rc):
                """gates -> (c, hf) for fwd columns [lo, lo+n).
                gate order: ig og fl fr r u  (sigmoid 0:20, tanh 20:24).
                Gate pre-activations are carried x GSCALE; the activation
                scale undoes it exactly."""
                g = sc_p.tile([128, 24, 65], F32, tag="gates", name="g")
                if ps is None:
                    nc.scalar.activation(g[:, 0:20, :n], PRE_F[:, 0:20, lo:lo + n],
                                         AF.Sigmoid, scale=GINV)
                    nc.scalar.activation(g[:, 20:24, :n], PRE_F[:, 20:24, lo:lo + n],
                                         AF.Tanh, scale=GINV)
                else:
                    nc.vector.tensor_add(g[:, :, :n], ps[:, 0:24, :n],
                                         PRE_F[:, 0:24, lo:lo + n])
                    nc.scalar.activation(g[:, 0:20, :n], g[:, 0:20, :n], AF.Sigmoid,
                                         scale=GINV)
                    nc.scalar.activation(g[:, 20:24, :n], g[:, 20:24, :n], AF.Tanh,
                                         scale=GINV)
                cnew = CF[:, :, lo:lo + n]
                t1 = sc_p.tile([128, 4, 65], F32, tag="t1", name="t1")
                t2 = sc_p.tile([128, 4, 65], F32, tag="t2", name="t2")
                # c = ig*u (+ fl*lc + fr*rc)
                nc.vector.tensor_mul(cnew, g[:, 0:4, :n], g[:, 20:24, :n])
                if lc is not None:
                    nc.vector.tensor_mul(t1[:, :, :n], g[:, 8:12, :n], lc)
                    nc.vector.tensor_add(cnew, cnew, t1[:, :, :n])
                    nc.vector.tensor_mul(t2[:, :, :n], g[:, 12:16, :n], rc)
                    nc.vector.tensor_add(cnew, cnew, t2[:, :, :n])
                # hf = og*tanh(c)*r + (1-r)*px = r*(hh - px) + px
                nc.scalar.activation(t1[:, :, :n], cnew, AF.Tanh)
                nc.vector.tensor_mul(t2[:, :, :n], g[:, 4:8, :n], t1[:, :, :n])  # hh
                px = PRE_F[:, 24:28, lo:lo + n]
                nc.vector.tensor_sub(t2[:, :, :n], t2[:, :, :n], px)
                nc.vector.tensor_mul(t2[:, :, :n], g[:, 16:20, :n], t2[:, :, :n])
                nc.vector.tensor_add(HF[:, :, lo:lo + n], t2[:, :, :n], px)

            def bwd_elem(lo, n, ps, pc):
                # gate order: ig og f r u  (sigmoid 0:16, tanh 16:20)
                g = sc_p.tile([128, 24, 65], F32, tag="gates", name="gb")
                if ps is None:
                    nc.scalar.activation(g[:, 0:16, :n], PRE_B[:, 0:16, lo:lo + n],
                                         AF.Sigmoid, scale=GINV)
                    nc.scalar.activation(g[:, 16:20, :n], PRE_B[:, 16:20, lo:lo + n],
                                         AF.Tanh, scale=GINV)
                else:
                    nc.vector.tensor_add(g[:, 0:20, :n], ps[:, 0:20, :n],
                                         PRE_B[:, 0:20, lo:lo + n])
                    nc.scalar.activation(g[:, 0:16, :n], g[:, 0:16, :n], AF.Sigmoid,
                                         scale=GINV)
                    nc.scalar.activation(g[:, 16:20, :n], g[:, 16:20, :n], AF.Tanh,
                                         scale=GINV)
                cnew = CB[:, :, lo:lo + n]
                t1 = sc_p.tile([128, 4, 65], F32, tag="t1", name="t1b")
                t2 = sc_p.tile([128, 4, 65], F32, tag="t2", name="t2b")
                nc.vector.tensor_mul(cnew, g[:, 0:4, :n], g[:, 16:20, :n])  # ig*u
                if pc is not None:
                    nc.vector.tensor_mul(t1[:, :, :n], g[:, 8:12, :n], pc)
                    nc.vector.tensor_add(cnew, cnew, t1[:, :, :n])
                nc.scalar.activation(t1[:, :, :n], cnew, AF.Tanh)
                nc.vector.tensor_mul(t2[:, :, :n], g[:, 4:8, :n], t1[:, :, :n])
                px = PRE_B[:, 20:24, lo:lo + n]
                nc.vector.tensor_sub(t2[:, :, :n], t2[:, :, :n], px)
                nc.vector.tensor_mul(t2[:, :, :n], g[:, 12:16, :n], t2[:, :, :n])
                nc.vector.tensor_add(HB[:, :, lo:lo + n], t2[:, :, :n], px)

            def fwd_gemm_step(lo, n, clo, wf_ovr=None):
                ch = sc_p.tile([128, 8, 65], F8, tag="ch", name="ch")
                lc = sc_p.tile([128, 4, 65], F32, tag="lc", name="lc")
                rc = sc_p.tile([128, 4, 65], F32, tag="rc", name="rc")
                nc.vector.tensor_scalar_mul(ch[:, 0:4, :n],
                                            HF[:, :, clo:clo + 2 * n - 1:2],
                                            HSCALE)
                nc.vector.tensor_copy(lc[:, :, :n], CF[:, :, clo:clo + 2 * n - 1:2])
                nc.vector.tensor_scalar_mul(ch[:, 4:8, :n],
                                            HF[:, :, clo + 1:clo + 2 * n:2],
                                            HSCALE)
                nc.vector.tensor_copy(rc[:, :, :n], CF[:, :, clo + 1:clo + 2 * n:2])
                ps = pr_p.tile([128, 24, 64], F32, tag="rps", name="ps")
                wf_sb = wf_ovr if wf_ovr is not None else cur["wf"]
                for m in range(24):
                    for k in range(8):
                        nc.tensor.matmul(ps[:, m, :n],
                                         wf_sb[:, (k * 24 + m) * 128:(k * 24 + m + 1) * 128],
                                         ch[:, k, :n],
                                         start=(k == 0), stop=(k == 7))
                fwd_elem(lo, n, ps, lc[:, :, :n], rc[:, :, :n])

            def bwd_gemm_step(lo, n, plo, after=None):
                ch = sc_p.tile([128, 8, 65], F8, tag="ch", name="chb")
                pc = sc_p.tile([128, 4, 65], F32, tag="lc", name="pcb")
                if after is not None:
                    # dependency injection: a throwaway write into ch that
                    # reads `after` holds this step (and the chain behind
                    # it) until `after` is produced — both in the
                    # scheduler's model and on hardware. Keeps the bwd tail
                    # inside the AllGather's latency window instead of
                    # being front-packed before the fwd chain ends.
                    nc.vector.tensor_scalar_mul(ch[:, 0:1, 0:1], after, HSCALE)
                if n == 1:
                    nc.vector.tensor_scalar_mul(ch[:, 0:4, 0:1],
                                                HB[:, :, plo:plo + 1], HSCALE)
                    nc.vector.tensor_copy(pc[:, :, 0:1], CB[:, :, plo:plo + 1])
                else:
                    m2 = n // 2
                    src_h = HB[:, :, plo:plo + m2].unsqueeze(3).broadcast_to(
                        [128, 4, m2, 2])
                    src_c = CB[:, :, plo:plo + m2].unsqueeze(3).broadcast_to(
                        [128, 4, m2, 2])
                    nc.vector.tensor_scalar_mul(
                        ch[:, 0:4, 0:n].rearrange("p c (a b) -> p c a b", b=2),
                        src_h, HSCALE)
                    nc.vector.tensor_copy(
                        pc[:, :, 0:n].rearrange("p c (a b) -> p c a b", b=2), src_c)
                ps = pr_p.tile([128, 24, 64], F32, tag="rps", name="psb")
                wb_sb = cur["wb"]
                for m in range(20):
                    for k in range(4):
                        nc.tensor.matmul(ps[:, m, :n],
                                         wb_sb[:, (k * 20 + m) * 128:(k * 20 + m + 1) * 128],
                                         ch[:, k, :n],
                                         start=(k == 0), stop=(k == 3))
                bwd_elem(lo, n, ps, pc[:, :, :n])

            def consume_gather(ccout, wf_sb, when_ms):
                """Gather-out DMAs + fwd top levels. The wait-until hint
                keeps the collective-gated DMAs from occupying SP-ring
                slots ahead of the next layer's weight stream."""
                with tc.tile_wait_until(when_ms):
                    for chn in range(4):
                        nc.sync.dma_start(
                            HF[:, chn, 135:143],
                            ccout[:, chn * 128:(chn + 1) * 128].rearrange(
                                "g p -> p g"))
                        nc.sync.dma_start(
                            CF[:, chn, 135:143],
                            ccout[:, 512 + chn * 128:512 + (chn + 1) * 128].rearrange(
                                "g p -> p g"))
                fwd_gemm_step(131, 4, 135, wf_ovr=wf_sb)   # top level 2
                fwd_gemm_step(129, 2, 131, wf_ovr=wf_sb)   # top level 1
                fwd_gemm_step(128, 1, 129, wf_ovr=wf_sb)   # root

            pending_gather = None
            cur = enqueue_weights(0, first=True)

            for l in range(L):
                bf_sb, bb_sb = cur["bf"], cur["bb"]
                ftile = cur["ft"]

                PRE_F = pre_p.tile([128, 28, NCOL], F32, tag="pref", name="PRE_F")
                PRE_B = pre_p.tile([128, 24, NCOL], F32, tag="preb", name="PRE_B")

                if pending_gather is not None:
                    consume_gather(*pending_gather)
                    pending_gather = None

                if l > 0:
                    for k in range(8):
                        src = HF if k < 4 else HB
                        nc.vector.tensor_copy(ftile[:, k, :], src[:, k % 4, :])

                # ---- pre-projections: PRE = W_pre @ feats (feature-major) ----
                # the PSUM->PRE moves alternate between the Act and DVE
                # engines so the 2-buffer psum rotation is reader-limited
                # by neither engine alone.
                for gidx in GORDER:
                    wpb = cur["wp"][gidx]
                    for mi in range(4):
                        m = gidx * 4 + mi
                        ps = pp_p.tile([128, 143], F32, tag="pps", name="pps")
                        for k in range(8):
                            nc.tensor.matmul(
                                ps[:],
                                wpb[:, (k * 4 + mi) * 128:(k * 4 + mi + 1) * 128],
                                ftile[:, k, :],
                                start=(k == 0), stop=(k == 7))
                        dst = (PRE_F[:, m, :] if m < 28
                               else PRE_B[:, m - 28, :])
                        bias = (bf_sb[:, m:m + 1] if m < 28
                                else bb_sb[:, m - 28:m - 27])
                        if mi % 2 == 0:
                            nc.scalar.activation(dst, ps[:], AF.Identity,
                                                 bias=bias)
                        else:
                            nc.vector.tensor_scalar_add(dst, ps[:], bias)

                # next layer's weight stream enqueues BEFORE the recurrence's
                # collective-dependent DMAs hit the SP ring
                nxt = enqueue_weights(l + 1, first=False) if l + 1 < L else None

                # ---- recurrences ----
                # fwd chain is the critical path to the AllGather; bwd steps
                # are interleaved so the PE can fill each chain's elementwise
                # latency with the other chain's matmuls.
                fwd_elem(63, 65, None, None, None)  # leaves (slots 63..127)
                bwd_elem(128, 1, None, None)        # root node 0
                # node-511 fix: slot 63 <- left child col 127 (masked), using
                # only the W_l half of wf (k-chunks 0..3). For cores != 0 the
                # mask zeroes the child, making this an idempotent leaf
                # recompute. Must run before the level-8 step below, which
                # consumes slot 63.
                chx = sc_p.tile([128, 8, 65], F8, tag="ch", name="chx")
                lcx = sc_p.tile([128, 4, 65], F32, tag="lc", name="lcx")
                rcx = sc_p.tile([128, 4, 65], F32, tag="rc", name="rcx")
                nc.vector.tensor_scalar(chx[:, 0:4, 0:1], HF[:, :, 127:128],
                                        HSCALE, mask_sb[:, 0:1],
                                        mybir.AluOpType.mult,
                                        mybir.AluOpType.mult)
                nc.vector.tensor_copy(lcx[:, :, 0:1], CF[:, :, 127:128])
                nc.vector.tensor_scalar_mul(lcx[:, :, 0:1], lcx[:, :, 0:1],
                                            mask_sb[:, 0:1])
                nc.vector.memset(rcx[:, :, 0:1], 0.0)
                psx = pr_p.tile([128, 24, 64], F32, tag="rps", name="psx")
                for m in range(24):
                    for k in range(4):
                        nc.tensor.matmul(
                            psx[:, m, 0:1],
                            cur["wf"][:, (k * 24 + m) * 128:(k * 24 + m + 1) * 128],
                            chx[:, k, 0:1], start=(k == 0), stop=(k == 3))
                fwd_elem(63, 1, psx, lcx[:, :, 0:1], rcx[:, :, 0:1])
                fwd_gemm_step(31, 32, 63)
                bwd_gemm_step(129, 2, 128)
                fwd_gemm_step(15, 16, 31)
                bwd_gemm_step(131, 4, 129)
                fwd_gemm_step(7, 8, 15)
                bwd_gemm_step(135, 8, 131)
                fwd_gemm_step(3, 4, 7)
                fwd_gemm_step(1, 2, 3)
                fwd_gemm_step(0, 1, 1)

                # AllGather the 8 subtree roots' (h, c)
                ccin = dram_p.tile([1024], F32, tag="ccin", name="ccin")
                ccout = dram_p.tile([8, 1024], F32, tag="ccout", name="ccout",
                                    addr_space="Shared")
                # flat small hint: below the natural model time for every
                # layer (a per-layer ramp turned into REAL sem-wait delays
                # for layer 1 -- pass-2 derives waits from scheduled ticks)
                with tc.tile_wait_until(0.04):
                    nc.sync.dma_start(
                        ccin[0:512].rearrange("(c p) -> p c", c=4, p=128),
                        HF[:, :, 0])
                    nc.sync.dma_start(
                        ccin[512:1024].rearrange("(c p) -> p c", c=4, p=128),
                        CF[:, :, 0])
                nc.gpsimd.collective_compute(
                    "AllGather", mybir.AluOpType.bypass,
                    ins=[ccin.opt()], outs=[ccout.opt()],
                    replica_groups=[list(range(NCORES))])
                pending_gather = (ccout, cur["wf"], 0.05)

                # rest of bwd chain (independent of the AllGather)
                # copy own root (col 135+c) into local slot 0
                tmp = sc_p.tile([128, 4, 8], F32, tag="pseltmp", name="pseltmp")
                pb = psel_sb[:, :].unsqueeze(1).broadcast_to([128, 4, 8])
                nc.vector.tensor_mul(tmp[:], HB[:, :, 135:143], pb)
                nc.vector.reduce_sum(HB[:, :, 0], tmp[:], mybir.AxisListType.X)
                tmp2 = sc_p.tile([128, 4, 8], F32, tag="pseltmp", name="pseltmp2")
                nc.vector.tensor_mul(tmp2[:], CB[:, :, 135:143], pb)
                nc.vector.reduce_sum(CB[:, :, 0], tmp2[:], mybir.AxisListType.X)
                bwd_gemm_step(1, 2, 0)
                bwd_gemm_step(3, 4, 1)
                bwd_gemm_step(7, 8, 3)
                bwd_gemm_step(15, 16, 7)
                bwd_gemm_step(31, 32, 15)
                bwd_gemm_step(63, 64, 31)
                bwd_gemm_step(127, 1, 63)    # node 1023

                if nxt is not None:
                    cur = nxt

            consume_gather(*pending_gather)

            # ---- outputs ----
            olv = out_loc_d[:].rearrange("(c p) n -> p c n", c=8, p=128)
            nc.sync.dma_start(olv[:, 0:4, :], HF[:, :, 0:128])
            nc.sync.dma_start(olv[:, 4:8, :], HB[:, :, 0:128])
            otv = out_top_d[:].rearrange("(c p) n -> p c n", c=8, p=128)
            nc.sync.dma_start(otv[:, 0:4, :], HF[:, :, 128:135])
            nc.sync.dma_start(otv[:, 4:8, :], HB[:, :, 128:135])

    nc.finalize()
    return nc


_program_cache = None


def kernel(features, f_px_w, f_px_b, f_x_w, f_x_b, f_l_w, f_l_b, f_r_w, f_r_b,
           b_px_w, b_px_b, b_x_w, b_x_b, b_h_w, b_h_b, left, right, parent):
    global _program_cache, _last_results
    features = np.asarray(features, dtype=np.float32)
    as32 = lambda a: np.asarray(a, dtype=np.float32)

    # ---- host-side packing (DRAM layout == SBUF layout, contiguous DMA) ----
    shared = {}
    for l in range(L):
        # gate rows of the pre-projections carry the x GSCALE fold (the px
        # highway rows stay raw)
        wpre = np.concatenate([_perm_f(as32(f_x_w[l])) * GSCALE, as32(f_px_w[l]),
                               _perm_b(as32(b_x_w[l])) * GSCALE, as32(b_px_w[l])],
                              axis=0)                    # [6656, 1024]
        t = _pack_lhsT(wpre, 8, 52, BF16)                # [8k, 52m, 128p, 128c]
        t = t.reshape(8, 13, 4, 128, 128).transpose(1, 3, 0, 2, 4)
        shared[f"wpre{l}"] = np.ascontiguousarray(t.reshape(13, 128, 4096))
        wrf = _perm_f(np.concatenate([as32(f_l_w[l]), as32(f_r_w[l])], axis=1))
        t = _pack_lhsT(wrf * WSCALE, 8, 24, FP8)         # [8, 24, 128, 128]
        shared[f"wrecf{l}"] = np.ascontiguousarray(
            t.transpose(2, 0, 1, 3).reshape(128, 8 * 24 * 128))
        t = _pack_lhsT(_perm_b(as32(b_h_w[l])) * WSCALE, 4, 20, FP8)
        shared[f"wrecb{l}"] = np.ascontiguousarray(
            t.transpose(2, 0, 1, 3).reshape(128, 4 * 20 * 128))
        bf = np.concatenate([_perm_f(as32(f_x_b[l]) + as32(f_l_b[l])
                                     + as32(f_r_b[l])) * GSCALE,
                             as32(f_px_b[l])])           # [3584]
        shared[f"biasf{l}"] = np.ascontiguousarray(bf.reshape(28, 128).T)
        bb = np.concatenate([_perm_b(as32(b_x_b[l]) + as32(b_h_b[l])) * GSCALE,
                             as32(b_px_b[l])])
        shared[f"biasb{l}"] = np.ascontiguousarray(bb.reshape(24, 128).T)

    in_maps = []
    ids_all = []
    for c in range(NCORES):
        ids = _node_ids(c)
        ids_all.append(ids)
        ft = features[ids].T.astype(BF16)                # [1024, 143]
        m = {k: v for k, v in shared.items()}
        m["featsT"] = np.ascontiguousarray(
            ft.reshape(8, 128, NCOL).transpose(1, 0, 2).reshape(128, 8 * NCOL))
        m["mask"] = np.full((128, 1), 1.0 if c == 0 else 0.0, np.float32)
        ps = np.zeros((128, 8), np.float32)
        ps[:, c] = 1.0
        m["psel"] = ps
        in_maps.append(m)

    if _program_cache is None:
        _program_cache = _build_program()
    nc = _program_cache

    trace = bool(os.environ.get("KERNEL_TRACE"))
    tdir = os.environ.get("KERNEL_TRACE_DIR") or None
    res = run_bass_kernel_spmd(nc, in_maps, core_ids=list(range(NCORES)),
                               trace=trace, tmpdir=tdir)
    _last_results = res

    out = np.empty((N, 2 * H), np.float32)
    for c in range(NCORES):
        loc = res.results[c]["out_loc"]                  # [1024, 128]
        nloc = 128 if c == 0 else 127
        out[ids_all[c][0:nloc]] = loc[:, 0:nloc].T
    out[0:7] = res.results[0]["out_top"].T
    return out
